# revision 1
# baseline (speedup 1.0000x reference)
"""Trainium2 Bass kernel for nn_MatrixAttention (sparse_attention).

Sharding: 8 cores = (batch b in 0..3) x (head-group g in 0..1, 4 heads each).
Each core: in_proj -> rcv conv (its 192 ch) -> row/col attention (4 heads)
-> pe conv -> grouped deconv (its 32 dc ch) -> partial final 3x3 conv over
all 64 output channels from its 32 dc channels. Host gather sums the pair
partials (input-dim-sharded conv => reduce-gather) and stacks batches.

Perf structure:
- Per-head prologue (scores/exp/Z/iz/V-permute) is emitted as generator
  steps interleaved into the previous head's combine loop, so PE-heavy
  score work overlaps the DVE/Pool-heavy combine.
- Raw-exp scores; the softmax normalizer 1/(Zr*Zc) is applied per pixel
  as the stt scalar (E-chunks) or the Act-drain scale (D-chunks).
- Combine chunks (128 pixels): PE matmul (ec^T V) -> E: DVE stt from
  PSUM, or D: Act drain to bf16 + DVE 2x tensor_tensor -> Pool half-fold
  (65->33 adds) -> DVE reduce-33 -> PE transpose -> batched Act copy
  into A (bf16).
- Zc via per-chunk ones-matmuls directly in chunk-partition layout; Zr
  via DVE free-axis reduce.
- pe-conv goes to a standalone P tensor (only needs v) interleaved into
  head 3; deconv accumulates dconv(A)+dconv(P) in PSUM; final 3x3 conv
  contracts 96-deep over a row-shifted dc3 (piecewise shift DMAs so S9
  pipelines behind S8).

Self-contained: hardcodes all shapes; no sibling imports.
"""
import sys
import numpy as np

sys.path.insert(0, "/opt/trn_rl_repo")

import ml_dtypes                        # noqa: E402
import concourse.bass as bass           # noqa: E402
import concourse.bacc as bacc           # noqa: E402
import concourse.mybir as mybir         # noqa: E402
from concourse.tile import TileContext  # noqa: E402
from concourse.bass_utils import run_bass_kernel_spmd  # noqa: E402
from concourse.alu_op_type import AluOpType  # noqa: E402

F32 = mybir.dt.float32
F32R = mybir.dt.float32r
BF16 = mybir.dt.bfloat16
AF = mybir.ActivationFunctionType
AX = mybir.AxisListType
BF16NP = ml_dtypes.bfloat16

NH, KD, HD = 8, 8, 16
SCALE = KD ** -0.5
H = 65            # spatial after in_proj
HP = 67           # padded
NPIX = H * H      # 4225
PADPIX = HP * HP  # 4489
IMG = 128
IMGP = 130
ID16 = 1040       # (i,d) = 65*16
NECS = 65 * 64    # 4160: w-major (h<64) ec storage


def r32(x):
    return x.bitcast(F32R)


def ap(tile, part0, nparts, free_off, free_dims):
    """AP over a tile: partitions [part0, part0+nparts), free offset + dims
    (list of [step, count], outer->inner)."""
    pitch = tile.ap[0][0]
    return bass.AP(tile.tensor, tile.offset + part0 * pitch + free_off,
                   [[pitch, nparts]] + [list(d) for d in free_dims])


# ----------------------------------------------------------------------------
# Host-side weight prep
# ----------------------------------------------------------------------------
def prep_core_inputs(inputs, b, g):
    inp = {k: np.ascontiguousarray(np.asarray(v), dtype=np.float32)
           for k, v in inputs.items()}
    heads = list(range(4 * g, 4 * g + 4))

    xp = np.zeros((64, IMGP, IMGP), np.float32)
    xp[:, 1:129, 1:129] = inp["x"][b]
    xp = xp.reshape(64, IMGP * IMGP)

    W1 = np.zeros((2, 2, 64, 128), np.float32)
    for co in range(128):
        W1[:, :, co // 2, co] = inp["w_in"][co, 0] * inp["s_in"][co]
    W1 = W1.reshape(4, 64, 128).transpose(1, 0, 2).reshape(64, 512)
    b1 = inp["b_in"].reshape(128, 1)

    # rcv conv weights. G1 (compact q): cols = [rq 4hx8 | rk | cq | ck].
    # G2 (v, padded): col 32*hi + dd  holds v-channel dd of head hi.
    w_rcv = inp["w_rcv"] * inp["s_rcv"][:, None, None, None]
    qrows = []
    for blk in range(4):           # rq, rk, cq, ck
        for h in heads:
            qrows.extend(range(h * 48 + blk * 8, h * 48 + blk * 8 + 8))
    Wq = w_rcv[qrows]              # [128, 128, 3, 3]
    bq = inp["b_rcv"][qrows].copy()
    scale_mask = np.ones(128, np.float32)
    scale_mask[0:32] = SCALE       # rq
    scale_mask[64:96] = SCALE      # cq
    Wq = Wq * scale_mask[:, None, None, None]
    bq = bq * scale_mask
    Wv = np.zeros((128, 128, 3, 3), np.float32)   # padded v rows
    bv = np.zeros((128, 1), np.float32)
    for hi, h in enumerate(heads):
        for dd in range(16):
            Wv[32 * hi + dd] = w_rcv[h * 48 + 32 + dd]
            bv[32 * hi + dd, 0] = inp["b_rcv"][h * 48 + 32 + dd]
    # lhsT [ci=128, 9 taps, 256 cols (G1 128 | G2 128)]
    Wrcv = np.concatenate(
        [Wq.transpose(1, 2, 3, 0).reshape(128, 9, 128),
         Wv.transpose(1, 2, 3, 0).reshape(128, 9, 128)], axis=2
    ).reshape(128, 9 * 256)
    brcv_g1 = bq.reshape(128, 1)
    brcv_g2 = bv

    # pe conv: input/output both padded to 128 (head hi at rows/cols 32*hi)
    w_pe = inp["w_pe"] * inp["s_pe"][:, None, None, None]
    Wpe = np.zeros((128, 3, 3, 128), np.float32)
    bpe = np.zeros((128, 1), np.float32)
    for hi, h_abs in enumerate(heads):
        for col in range(16):
            co = h_abs * 16 + col
            col_l = 32 * hi + col
            for k in range(2):
                ci_row = 32 * hi + 2 * (col // 2) + k
                Wpe[ci_row, :, :, col_l] = w_pe[co, k]
            bpe[col_l, 0] = inp["b_pe"][co]
    Wpe = Wpe.reshape(128, 9 * 128)

    w_dc = inp["w_dc"]
    g0 = heads[0] * 8
    Wdc = np.zeros((128, 2, 2, 32), np.float32)   # rows = padded A channels
    bdc = np.zeros((32, 1), np.float32)
    for cl in range(32):
        co = g0 + cl
        hi, c = cl // 8, cl % 8
        for k in range(2):
            Wdc[32 * hi + 2 * c + k, :, :, cl] = w_dc[co, k]
        bdc[cl, 0] = inp["b_dc"][co]
    Wdc = Wdc.reshape(128, 4 * 32)

    # final conv, 96-deep (ky folded into contraction): rows (ky, ci32),
    # cols (kx, co64)
    w_out = inp["w_out"] * inp["s_out"][:, None, None, None]   # [64,64,3,3]
    Wout3 = np.zeros((96, 3, 64), np.float32)
    for ky in range(3):
        for ci in range(32):
            for kx in range(3):
                Wout3[ky * 32 + ci, kx, :] = w_out[:, 32 * g + ci, ky, kx]
    Wout3 = Wout3.reshape(96, 192)
    bfin = (inp["b_out"] if g == 0 else np.zeros(64, np.float32)).reshape(64, 1)

    return {
        "xp": xp.astype(BF16NP), "W1": np.ascontiguousarray(W1).astype(BF16NP), "b1": b1,
        "Wrcv": np.ascontiguousarray(Wrcv).astype(BF16NP),
        "brcv_g1": brcv_g1, "brcv_g2": brcv_g2,
        "Wpe": np.ascontiguousarray(Wpe), "bpe": bpe,
        "Wdc": np.ascontiguousarray(Wdc).astype(BF16NP), "bdc": bdc,
        "Wout3": np.ascontiguousarray(Wout3).astype(BF16NP), "bfin": bfin,
        "ident": np.eye(128, dtype=np.float32),
        "ones": np.ones((65, 2), np.float32),
        "zeros": np.zeros((128, PADPIX), np.float32),
    }


# ----------------------------------------------------------------------------
# Device program
# ----------------------------------------------------------------------------
def build_nc():
    nc = bacc.Bacc(None, target_bir_lowering=False)

    dins = {}
    for name, shape, dt_ in [
        ("xp", [64, IMGP * IMGP], BF16), ("W1", [64, 512], BF16),
        ("b1", [128, 1], F32),
        ("Wrcv", [128, 2304], BF16), ("brcv_g1", [128, 1], F32),
        ("brcv_g2", [128, 1], F32),
        ("Wpe", [128, 1152], F32R), ("bpe", [128, 1], F32),
        ("Wdc", [128, 128], BF16), ("bdc", [32, 1], F32),
        ("Wout3", [96, 192], BF16), ("bfin", [64, 1], F32),
        ("ident", [128, 128], F32R),
        ("ones", [65, 2], F32R),
        ("zeros", [128, PADPIX], F32R),
    ]:
        dins[name] = nc.dram_tensor(name, shape, dt_, kind="ExternalInput")
    out_d = nc.dram_tensor("out", [64, IMG, IMG], BF16, kind="ExternalOutput")
    zbf = dins["zeros"].bitcast(BF16)   # [128, 2*PADPIX] of bf16 zeros

    with TileContext(nc) as tc:
        with (
            tc.tile_pool(name="wpool", bufs=1) as wp,
            tc.tile_pool(name="vpool", bufs=1) as vp_,
            tc.tile_pool(name="apool", bufs=1) as ap_,
        ):
            def load(name, shape, dt_=F32):
                t = wp.tile(shape, dt_, tag=name)
                # big weight tensors go on the Act DGE queue so the x/W1
                # loads on the SP queue start immediately
                eng = nc.scalar if shape[0] * shape[1] > 4096 else nc.sync
                eng.dma_start(out=t[:, :], in_=dins[name][:, :])
                return t

            Wrcv = load("Wrcv", [128, 2304], BF16)
            brg1 = load("brcv_g1", [128, 1])
            brg2 = load("brcv_g2", [128, 1])
            Wpe = load("Wpe", [128, 1152], F32R)
            bpe = load("bpe", [128, 1])
            Wdc = load("Wdc", [128, 128], BF16)
            bdc = load("bdc", [32, 1])
            Wout3 = load("Wout3", [96, 192], BF16)
            bfin = load("bfin", [64, 1])
            ident = load("ident", [128, 128], F32R)
            ones65 = load("ones", [65, 2], F32R)

            v_sb = vp_.tile([128, PADPIX + 2 * HP], F32R, tag="v")  # (h,w) pad
            nc.gpsimd.memset(v_sb[:, :].bitcast(F32), 0.0)
            A_sb = ap_.tile([128, NPIX], BF16, tag="A")      # (w,h)-major
            P_sb = ap_.tile([128, NPIX], BF16, tag="P")      # pe-conv out
            # zero only the pad rows (16-31 of each 32-row head block)
            for hi in range(4):
                nc.sync.dma_start(out=A_sb[32 * hi + 16:32 * hi + 32, :],
                                  in_=zbf[:16, :NPIX])

            with tc.tile_pool(name="qxpool", bufs=1) as qx:
                qQ = qx.tile([128, NPIX + H], F32R, tag="qQ")
                qK = qx.tile([128, NPIX + H], F32R, tag="qK")
                qC1 = qx.tile([128, NPIX + H], F32R, tag="qC1")
                qC2 = qx.tile([128, NPIX + H], F32R, tag="qC2")
                for _t in (qQ, qK, qC1, qC2):
                    nc.gpsimd.memset(_t[:, NPIX:].bitcast(F32), 0.0)

                with tc.tile_pool(name="ypool", bufs=1) as yp:
                    y_sb = yp.tile([128, PADPIX + 2 * HP + 1], BF16, tag="y")
                    nc.gpsimd.memset(y_sb[:, :].bitcast(F32), 0.0)

                    # ===== S1: in_proj (x loaded in two halves) =====
                    with (
                        tc.tile_pool(name="xpool", bufs=2) as xp_pool,
                        tc.tile_pool(name="ps1", bufs=2, space="PSUM") as ps1,
                    ):
                        W1 = xp_pool.tile([64, 512], BF16, tag="w1")
                        nc.sync.dma_start(out=W1[:, :], in_=dins["W1"][:, :])
                        b1 = xp_pool.tile([128, 1], F32, tag="b1")
                        nc.sync.dma_start(out=b1[:, :], in_=dins["b1"][:, :])

                        chunks = [(0, 7), (7, 7), (14, 7), (21, 7), (28, 4),
                                  (32, 7), (39, 7), (46, 7), (53, 7), (60, 5)]
                        for half in range(2):
                            xt = xp_pool.tile([64, 68 * IMGP], BF16, tag="x")
                            src_off = 0 if half == 0 else 64 * IMGP
                            nc.sync.dma_start(
                                out=xt[:, :33 * IMGP],
                                in_=dins["xp"][:, src_off:src_off + 33 * IMGP])
                            nc.scalar.dma_start(
                                out=xt[:, 33 * IMGP:66 * IMGP],
                                in_=dins["xp"][:, src_off + 33 * IMGP:
                                               src_off + 66 * IMGP])
                            nc.gpsimd.memset(xt[:, 66 * IMGP:].bitcast(F32), 0.0)
                            row0 = 0 if half == 0 else 64
                            for c0, nr in chunks:
                                if (half == 0) != (c0 < 32):
                                    continue
                                pt = ps1.tile([128, 7 * 66], F32, tag="ps1")
                                for t, (ky, kx) in enumerate(
                                        [(0, 0), (0, 1), (1, 0), (1, 1)]):
                                    rhs = ap(xt, 0, 64,
                                             (2 * c0 + ky - row0) * IMGP + kx,
                                             [[2 * IMGP, nr], [2, 66]])
                                    nc.tensor.matmul(
                                        pt[:, :nr * 66],
                                        W1[:, t * 128:(t + 1) * 128],
                                        rhs, start=(t == 0), stop=(t == 3))
                                dst = ap(y_sb, 0, 128, (c0 + 1) * HP + 1,
                                         [[HP, nr], [1, H]])
                                nc.scalar.activation(dst,
                                                     ap(pt, 0, 128, 0,
                                                        [[66, nr], [1, H]]),
                                                     AF.Identity, bias=b1[:, :])

                    # ===== S2: rcv conv (q compact bf16 + v padded f32r) ====
                    with (
                        tc.tile_pool(name="qcpool", bufs=1) as qcp,
                        tc.tile_pool(name="ps2", bufs=2, space="PSUM") as ps2,
                    ):
                        q_sb = qcp.tile([128, NPIX], F32R, tag="qc")
                        for c0 in range(0, H, 7):
                            nr = min(7, H - c0)
                            pt = ps2.tile([128, 7 * 66], F32, tag="ps2")
                            for t in range(9):
                                ky, kx = t // 3, t % 3
                                rhs = ap(y_sb, 0, 128, (c0 + ky) * HP + kx,
                                         [[HP, nr], [1, 66]])
                                nc.tensor.matmul(
                                    pt[:, :nr * 66],
                                    Wrcv[:, t * 256:t * 256 + 128],
                                    rhs, start=(t == 0), stop=(t == 8))
                            nc.scalar.activation(q_sb[:, c0 * H:(c0 + nr) * H],
                                                 ap(pt, 0, 128, 0,
                                                    [[66, nr], [1, H]]),
                                                 AF.Identity, bias=brg1[:, :])
                            pt2 = ps2.tile([128, 7 * 66], F32, tag="ps2")
                            for t in range(9):
                                ky, kx = t // 3, t % 3
                                rhs = ap(y_sb, 0, 128, (c0 + ky) * HP + kx,
                                         [[HP, nr], [1, 66]])
                                nc.tensor.matmul(
                                    pt2[:, :nr * 66],
                                    Wrcv[:, t * 256 + 128:t * 256 + 256],
                                    rhs, start=(t == 0), stop=(t == 8))
                            dstv = ap(v_sb, 0, 128, (c0 + 1) * HP + 1,
                                      [[HP, nr], [1, H]])
                            nc.scalar.activation(dstv,
                                                 ap(pt2, 0, 128, 0,
                                                    [[66, nr], [1, H]]),
                                                 AF.Identity, bias=brg2[:, :])
                        # reshuffle q -> 32-aligned padded tensors (sbuf
                        # dma, spread across DGE queues to parallelize issue)
                        qeng = [nc.sync, nc.scalar]
                        for hi in range(4):
                            for blk, dstq in enumerate([qQ, qK, qC1, qC2]):
                                qeng[(hi * 4 + blk) % 2].dma_start(
                                    out=ap(dstq, 32 * hi, 8, 0, [[1, NPIX]]),
                                    in_=q_sb[blk * 32 + 8 * hi:
                                             blk * 32 + 8 * hi + 8, :])

                # ===== S3-S6: attention, software-pipelined per head =====
                # Per-head prologue (scores/exp/Z/iz/V-permute) is emitted as
                # generator steps interleaved into the PREVIOUS head's chunk
                # loop, so PE-heavy score work overlaps DVE/Pool-heavy chunks.
                with (
                    tc.tile_pool(name="hpool", bufs=2) as hp,
                    tc.tile_pool(name="mpool", bufs=3) as mp,
                    tc.tile_pool(name="up2pool", bufs=3) as up2,
                    tc.tile_pool(name="mhpool", bufs=2) as mhp,
                    tc.tile_pool(name="tpool", bufs=6) as tp,
                    tc.tile_pool(name="scps", bufs=2, space="PSUM") as scps,
                    tc.tile_pool(name="ups", bufs=2, space="PSUM") as ups,
                ):
                    zero16 = mp.tile([128, 16], BF16, tag="z16")
                    nc.sync.dma_start(out=zero16[:, :], in_=zbf[:, :16])

                    def alloc_head():
                        t = {}
                        for nm, shape, dt_ in [
                            ("er", [65, NPIX], BF16),
                            ("er2", [128, NPIX], BF16),
                            ("ertail", [65, 65], BF16),
                            ("ecs", [65, NECS], F32R),
                            ("ectail", [65, 65], F32R),
                            ("zravg", [65, 65], F32),
                            ("zrc", [128, 34], F32),
                            ("zcc", [128, 34], F32),
                            ("iz2", [128, 34], F32),
                            ("vpt", [65, ID16], F32R),
                        ]:
                            tl = hp.tile(shape, dt_, tag=nm)
                            t[nm] = tl
                        return t

                    def prologue_steps(hi, t):
                        """Generator: emits one instruction group per next()."""
                        tpos = (32 * hi, 0)
                        er, ecs, ectail = t["er"], t["ecs"], t["ectail"]
                        # V-permute first (only needs v_sb)
                        for i0 in range(0, H, 32):
                            ni = min(32, H - i0)
                            ptv = scps.tile([128, 512], F32, tag="sc")
                            for k in range(ni):
                                i = i0 + k
                                src = ap(v_sb, 32 * hi, 16,
                                         (i + 1) * HP + 1, [[1, H]])
                                idn = ap(ident, 32 * hi, 16, 32 * hi,
                                         [[1, 16]])
                                nc.tensor.transpose(
                                    r32(ap(ptv, 0, 65, k * 16, [[1, 16]])),
                                    src, idn, tile_position=tpos)
                            nc.scalar.activation(
                                ap(t["vpt"], 0, 65, i0, [[1, ni], [H, 16]]),
                                ptv[:65, :ni * 16], AF.Copy)
                            yield
                        # r scores: per w -> psum [h, i]; exp -> er (bf16)
                        for w0 in range(0, H, 7):
                            nw = min(7, H - w0)
                            pt = scps.tile([128, 512], F32, tag="sc")
                            for k in range(nw):
                                w = w0 + k
                                nc.tensor.matmul(
                                    pt[:65, k * 66:k * 66 + 66],
                                    ap(qK, 32 * hi, 8, w, [[H, H]]),
                                    ap(qQ, 32 * hi, 8, w, [[H, 66]]),
                                    start=True, stop=True,
                                    tile_position=tpos)
                            nc.scalar.activation(er[:, w0 * H:(w0 + nw) * H],
                                                 ap(pt, 0, 65, 0,
                                                    [[66, nw], [1, H]]),
                                                 AF.Exp)
                            yield
                        # er2 (pixel-partitioned) + ertail + zr
                        nc.sync.dma_start(out=t["er2"][0:64, :],
                                          in_=er[0:64, :])
                        nc.sync.dma_start(out=t["er2"][64:128, :NPIX - H],
                                          in_=er[0:64, H:])
                        nc.sync.dma_start(out=t["ertail"][:, :],
                                          in_=ap(er, 64, 1, 0,
                                                 [[H, H], [1, H]]))
                        nc.vector.tensor_reduce(
                            t["zravg"][:, :],
                            ap(er, 0, 65, 0, [[H, H], [1, H]]),
                            AX.X, AluOpType.add)
                        yield
                        # c scores: per h -> psum [j, w]; exp -> ecs (f32r)
                        for h0 in range(0, H, 7):
                            nh = min(7, H - h0)
                            pt = scps.tile([128, 512], F32, tag="sc")
                            for k in range(nh):
                                h = h0 + k
                                nc.tensor.matmul(
                                    pt[:65, k * 66:k * 66 + 66],
                                    ap(qC1, 32 * hi, 8, h * H, [[1, H]]),
                                    ap(qC2, 32 * hi, 8, h * H, [[1, 66]]),
                                    start=True, stop=True,
                                    tile_position=tpos)
                            nhs = min(nh, 64 - h0)
                            nc.scalar.activation(
                                ap(ecs, 0, 65, h0, [[1, nhs], [64, H]]),
                                ap(pt, 0, 65, 0, [[66, nhs], [1, H]]),
                                AF.Exp)
                            if h0 + nh == 65:
                                nc.scalar.activation(
                                    ectail[:, :],
                                    ap(pt, 0, 65, (nh - 1) * 66, [[1, H]]),
                                    AF.Exp)
                            yield
                        # Zc per chunk (chunk-partitioned ones-matmuls)
                        zct = scps.tile([128, 512], F32, tag="sc")
                        for wb0 in range(0, 32, 16):
                            for wb in range(wb0, wb0 + 16):
                                nc.tensor.matmul(
                                    ap(zct, 0, 128, 2 * wb, [[1, 2]]),
                                    ap(ecs, 0, 65, wb * 128, [[1, 128]]),
                                    ones65[:, :], start=True, stop=True)
                            yield
                        nc.tensor.matmul(
                            ap(zct, 0, 64, 64, [[1, 2]]),
                            ap(ecs, 0, 65, 64 * 64, [[1, 64]]),
                            ones65[:, :], start=True, stop=True)
                        nc.tensor.matmul(
                            ap(zct, 0, 65, 66, [[1, 2]]),
                            ap(ectail, 0, 65, 0, [[1, H]]),
                            ones65[:, :], start=True, stop=True)
                        zcc = t["zcc"]
                        nc.scalar.activation(zcc[0:64, :],
                                             ap(zct, 0, 64, 0, [[2, 34]]),
                                             AF.Copy)
                        nc.scalar.activation(zcc[64:128, 0:32],
                                             ap(zct, 64, 64, 0, [[2, 32]]),
                                             AF.Copy)
                        nc.scalar.activation(zcc[64:65, 33:34],
                                             ap(zct, 64, 1, 66, [[1, 1]]),
                                             AF.Copy)
                        yield
                        # zr chunk columns + iz scalars
                        zravg, zrc = t["zravg"], t["zrc"]
                        nc.vector.tensor_copy(
                            ap(zrc, 0, 64, 0, [[1, 32]]),
                            ap(zravg, 0, 64, 0, [[2, 32]]))
                        nc.vector.tensor_copy(
                            ap(zrc, 64, 64, 0, [[1, 32]]),
                            ap(zravg, 0, 64, 1, [[2, 32]]))
                        nc.vector.tensor_copy(zrc[0:64, 32:33],
                                              zravg[0:64, 64:65])
                        nc.sync.dma_start(out=zrc[0:65, 33:34],
                                          in_=ap(zravg, 64, 1, 0, [[1, H]]))
                        iz2 = t["iz2"]
                        nc.vector.tensor_tensor(out=iz2[:, :], in0=zrc[:, :],
                                                in1=zcc[:, :],
                                                op=AluOpType.mult)
                        nc.vector.reciprocal(iz2[:, :], iz2[:, :])
                        yield

                    def pe_p_steps():
                        """S7 pe-conv into standalone P (only needs v_sb)."""
                        for w0 in range(0, H, 7):
                            nw = min(7, H - w0)
                            pt = scps.tile([128, 512], F32, tag="sc")
                            for tt in range(9):
                                ky, kx = tt // 3, tt % 3
                                rhs = ap(v_sb, 0, 128, ky * HP + kx + w0,
                                         [[1, nw], [HP, 66]])
                                nc.tensor.matmul(
                                    pt[:, :nw * 66],
                                    Wpe[:, tt * 128:tt * 128 + 128],
                                    rhs, start=(tt == 0), stop=(tt == 8))
                            nc.scalar.activation(
                                P_sb[:, w0 * H:(w0 + nw) * H],
                                ap(pt, 0, 128, 0, [[66, nw], [1, H]]),
                                AF.Identity, bias=bpe[:, :])
                            yield

                    state = {"ptt": None, "off": 0}

                    def do_chunk(t, idx, lhsT_ap, er_ap, izcol, M):
                        ut = ups.tile([128, ID16], F32, tag="ut")
                        for n0 in (0, 512, 1024):
                            nn = min(512, ID16 - n0)
                            nc.tensor.matmul(ut[:M, n0:n0 + nn],
                                             lhsT_ap,
                                             t["vpt"][:, n0:n0 + nn],
                                             start=True, stop=True)
                        on_e = (idx % 4 == 0)
                        m = mp.tile([128, ID16], BF16, tag="m")
                        if on_e:
                            # DVE: (ut * iz) * er straight out of PSUM (1x)
                            nc.vector.scalar_tensor_tensor(
                                out=ap(m, 0, M, 0, [[65, 16], [1, 65]]),
                                in0=ap(ut, 0, M, 0, [[65, 16], [1, 65]]),
                                scalar=t["iz2"][:M, izcol:izcol + 1],
                                in1=er_ap,
                                op0=AluOpType.mult, op1=AluOpType.mult)
                        else:
                            # Act drains PSUM to bf16 applying iz via scale;
                            # DVE multiplies by raw er at 2x
                            utb = up2.tile([128, ID16], BF16, tag="utb")
                            nc.scalar.activation(
                                ap(utb, 0, M, 0, [[65, 16], [1, 65]]),
                                ap(ut, 0, M, 0, [[65, 16], [1, 65]]),
                                AF.Identity,
                                scale=t["iz2"][:M, izcol:izcol + 1])
                            nc.vector.tensor_tensor(
                                out=ap(m, 0, M, 0, [[65, 16], [1, 65]]),
                                in0=ap(utb, 0, M, 0, [[65, 16], [1, 65]]),
                                in1=er_ap, op=AluOpType.mult)
                        # Pool half-fold 65 -> 33 (32 pair-sums + i=64 tail)
                        mh = mhp.tile([128, 528], BF16, tag="mh")
                        nc.gpsimd.tensor_tensor(
                            out=ap(mh, 0, M, 0, [[33, 16], [1, 32]]),
                            in0=ap(m, 0, M, 0, [[65, 16], [1, 32]]),
                            in1=ap(m, 0, M, 32, [[65, 16], [1, 32]]),
                            op=AluOpType.add)
                        nc.gpsimd.tensor_tensor(
                            out=ap(mh, 0, M, 32, [[33, 16], [1, 1]]),
                            in0=ap(m, 0, M, 64, [[65, 16], [1, 1]]),
                            in1=ap(zero16, 0, M, 0, [[0, 16], [1, 1]]),
                            op=AluOpType.add)
                        at = tp.tile([128, 16], F32, tag="at")
                        nc.vector.tensor_reduce(
                            at[:M, :], ap(mh, 0, M, 0, [[33, 16], [1, 33]]),
                            AX.X, AluOpType.add)
                        if state["ptt"] is None:
                            pttt = scps.tile([128, 512], F32, tag="sc")
                            state["ptt"] = pttt
                            state["off"] = 0
                        nc.tensor.transpose(
                            ap(state["ptt"], 0, 16, state["off"], [[1, M]]),
                            at[:M, :], ident[:M, :M].bitcast(F32))
                        state["off"] += M

                    def flush(dst_ap):
                        nc.scalar.activation(
                            dst_ap,
                            ap(state["ptt"], 0, 16, 0, [[1, state["off"]]]),
                            AF.Copy)
                        state["ptt"] = None

                    tiles = alloc_head()
                    for _ in prologue_steps(0, tiles):
                        pass
                    for hi in range(4):
                        A0 = 32 * hi
                        t = tiles
                        if hi < 3:
                            tiles = alloc_head()
                            nxt = prologue_steps(hi + 1, tiles)
                        else:
                            nxt = pe_p_steps()
                        er, ertail = t["er"], t["ertail"]
                        for wb in range(32):
                            do_chunk(t, wb,
                                     ap(t["ecs"], 0, 65, wb * 128, [[1, 128]]),
                                     ap(t["er2"], 0, 128, 2 * wb * H,
                                        [[0, 16], [1, H]]),
                                     wb, 128)
                            if wb % 4 == 3:
                                wb0 = wb - 3
                                flush(ap(A_sb, A0, 16, 2 * wb0 * H,
                                         [[H, 8], [1, 64]]))
                            next(nxt, None)
                        do_chunk(t, 32,
                                 ap(t["ecs"], 0, 65, 64 * 64, [[1, 64]]),
                                 ap(t["er2"], 0, 64, 64 * H,
                                    [[0, 16], [1, H]]),
                                 32, 64)
                        flush(ap(A_sb, A0, 16, 64 * H, [[1, 64]]))
                        next(nxt, None)
                        do_chunk(t, 33,
                                 ap(t["ectail"], 0, 65, 0, [[1, H]]),
                                 ap(ertail, 0, 65, 0, [[0, 16], [1, H]]),
                                 33, 65)
                        flush(ap(A_sb, A0, 16, 64, [[H, H]]))
                        for _ in nxt:
                            pass

            # ===== S8: dconv(A) + dconv(P) -> dc3 rows 0:32 =====
            with (
                tc.tile_pool(name="dcpool", bufs=1) as dcp,
                tc.tile_pool(name="ps8", bufs=2, space="PSUM") as ps8,
            ):
                dc3 = dcp.tile([96, IMGP * IMGP], BF16, tag="dc3")
                # zero borders: block b holds dcpad rows shifted by b, so
                # block0 rows {0,129}, block1 rows {128,129-ish}, block2
                # rows {127,128}; plus the 1-px column strips everywhere.
                nc.sync.dma_start(out=ap(dc3, 0, 32, 0, [[1, IMGP]]),
                                  in_=zbf[:32, :IMGP])
                nc.sync.dma_start(
                    out=ap(dc3, 0, 32, 129 * IMGP, [[1, IMGP]]),
                    in_=zbf[:32, :IMGP])
                nc.sync.dma_start(
                    out=ap(dc3, 32, 32, 128 * IMGP, [[1, 2 * IMGP]]),
                    in_=zbf[:32, :2 * IMGP])
                nc.sync.dma_start(
                    out=ap(dc3, 64, 32, 127 * IMGP, [[1, 3 * IMGP]]),
                    in_=zbf[:32, :3 * IMGP])
                for blk in range(3):
                    nc.sync.dma_start(
                        out=ap(dc3, 32 * blk, 32, IMGP, [[IMGP, 128], [1, 1]]),
                        in_=zbf[:32, :128])
                    nc.sync.dma_start(
                        out=ap(dc3, 32 * blk, 32, IMGP + 129,
                               [[IMGP, 128], [1, 1]]),
                        in_=zbf[:32, :128])
                # a0-outer so dc rows complete in ascending order; the
                # row-shifted copies for the 96-deep final conv are issued
                # piecewise so S9 can pipeline behind S8.
                shift_done = 0

                def dc3_shift_upto(row):
                    nonlocal shift_done
                    lo = shift_done
                    if row <= lo:
                        return
                    nc.sync.dma_start(
                        out=dc3[32:64, lo * IMGP:row * IMGP],
                        in_=dc3[0:32, (lo + 1) * IMGP:(row + 1) * IMGP])
                    nc.scalar.dma_start(
                        out=dc3[64:96, lo * IMGP:row * IMGP],
                        in_=dc3[0:32, (lo + 2) * IMGP:(row + 2) * IMGP])
                    shift_done = row

                for a0 in range(0, 64, 8):
                    for pr in range(2):
                        for ps in range(2):
                            pt = ps8.tile([32, 512], F32, tag="dcps")
                            w0 = (pr * 2 + ps) * 32
                            nc.tensor.matmul(
                                pt[:, :], Wdc[:, w0:w0 + 32],
                                ap(A_sb, 0, 128, ps * H + pr + a0,
                                   [[1, 8], [H, 64]]),
                                start=True, stop=False)
                            nc.tensor.matmul(
                                pt[:, :], Wdc[:, w0:w0 + 32],
                                ap(P_sb, 0, 128, ps * H + pr + a0,
                                   [[1, 8], [H, 64]]),
                                start=False, stop=True)
                            dst = ap(dc3, 0, 32,
                                     (2 * a0 + pr + 1) * IMGP + ps + 1,
                                     [[2 * IMGP, 8], [2, 64]])
                            nc.scalar.activation(dst, pt[:, :], AF.Identity,
                                                 bias=bdc[:, :])
                    if a0 in (24, 40, 56):
                        # rows complete up to 2*a0+16 after this block
                        dc3_shift_upto(2 * a0 + 14)
                nc.sync.dma_start(
                    out=dc3[32:64, shift_done * IMGP:IMGP * IMGP - IMGP],
                    in_=dc3[0:32, (shift_done + 1) * IMGP:])
                nc.scalar.dma_start(
                    out=dc3[64:96, shift_done * IMGP:IMGP * IMGP - 2 * IMGP],
                    in_=dc3[0:32, (shift_done + 2) * IMGP:IMGP * IMGP])

                # ===== S9: final conv partial, 96-deep =====
                with (
                    tc.tile_pool(name="opool", bufs=4) as op_,
                    tc.tile_pool(name="ps9", bufs=2, space="PSUM") as ps9,
                ):
                    for r0 in range(0, IMG, 4):
                        pt = ps9.tile([64, 512], F32, tag="o")
                        for kx in range(3):
                            rhs = ap(dc3, 0, 96, r0 * IMGP + kx,
                                     [[IMGP, 4], [1, IMG]])
                            nc.tensor.matmul(pt[:, :],
                                             Wout3[:, kx * 64:kx * 64 + 64],
                                             rhs, start=(kx == 0),
                                             stop=(kx == 2))
                        ost = op_.tile([64, 512], BF16, tag="ost")
                        nc.scalar.activation(ost[:, :], pt[:, :], AF.Identity,
                                             bias=bfin[:, :])
                        oeng = nc.sync if (r0 // 4) % 2 == 0 else nc.scalar
                        oeng.dma_start(out=out_d[:, r0:r0 + 4, :],
                                       in_=ost[:, :])

    nc.compile()
    return nc


_NC_CACHE = None


def kernel(**inputs):
    global _NC_CACHE
    if _NC_CACHE is None:
        _NC_CACHE = build_nc()
    nc = _NC_CACHE
    in_maps = [prep_core_inputs(inputs, c // 2, c % 2) for c in range(8)]
    res = run_bass_kernel_spmd(nc, in_maps, list(range(8)))
    out = np.zeros((4, 64, IMG, IMG), np.float32)
    for b in range(4):
        out[b] = (res.results[2 * b]["out"].astype(np.float32) +
                  res.results[2 * b + 1]["out"].astype(np.float32))
    return out



# revision 28
# speedup vs baseline: 1.0972x; 1.0972x over previous
"""Trainium2 Bass kernel for nn_MatrixAttention (sparse_attention).

Sharding: 8 cores = (batch b in 0..3) x (head-group g in 0..1, 4 heads each).
Each core: in_proj -> rcv conv (its 192 ch) -> row/col attention (4 heads)
-> pe conv -> grouped deconv (its 32 dc ch) -> partial final 3x3 conv over
all 64 output channels from its 32 dc channels. Host gather sums the pair
partials (input-dim-sharded conv => reduce-gather) and stacks batches.

Perf structure (TimelineSim-tuned):
- q/ecs/vpt/scores all bf16 (f32r matmuls with <256-col outputs pay a 4x
  cycle penalty; bf16 is 1 cycle/row and halves SBUF).
- S2 split: G1 (q) chunks first, then one merged q-reshuffle (sync+gpsimd
  DGE queues; the ~630ns/DMA descriptor-gen serializes on HWDGE, so it
  must never sit on the Act/SP queues mid-pipeline), then G2 (v) chunks
  with the head-0 prologue generator interleaved (scores/exp/Zc/Zr/iz
  hide behind G2's PE work; V-permute last since it needs v).
- Combine chunks (128 px): PE matmul (ec^T V, 65-deep, 1040 cols) ->
  D-chunks: Act drain (iz scale) to bf16 + DVE tensor_tensor x er (2x) |
  E-chunks (1 in 4): DVE stt from PSUM (1x) -> Pool fold 65->33 ->
  DVE reduce-33 -> PE transpose -> Act flush into A.
  The fold2/reduce of chunk N runs at the TOP of chunk N+2 (pend2) and
  transposes/flushes are deferred 4 chunks (pend): in-order engine queues
  otherwise serialize the whole chain per chunk.
- Zr via Pool fold (w,66-stride er, pad col zeroed per head) + DVE
  reduce-33; Zc via ones-matmuls; iz=1/(Zr*Zc) as drain scale/stt scalar.
- S8/S9 drains alternate Act / DVE-stt(+bias broadcast); output stores
  batched 4 row-groups per DMA (HWDGE descriptor-gen is the tail limit).

Self-contained: hardcodes all shapes; no sibling imports.
"""
import sys
import numpy as np

sys.path.insert(0, "/opt/trn_rl_repo")

import ml_dtypes                        # noqa: E402
import concourse.bass as bass           # noqa: E402
import concourse.bacc as bacc           # noqa: E402
import concourse.mybir as mybir         # noqa: E402
from concourse.tile import TileContext  # noqa: E402
from concourse.bass_utils import run_bass_kernel_spmd  # noqa: E402
from concourse.alu_op_type import AluOpType  # noqa: E402

F32 = mybir.dt.float32
F32R = mybir.dt.float32r
BF16 = mybir.dt.bfloat16
AF = mybir.ActivationFunctionType
AX = mybir.AxisListType
BF16NP = ml_dtypes.bfloat16

NH, KD, HD = 8, 8, 16
SCALE = KD ** -0.5
H = 65            # spatial after in_proj
HP = 67           # padded
NPIX = H * H      # 4225
PADPIX = HP * HP  # 4489
IMG = 128
IMGP = 130
ID16 = 1040       # (i,d) = 65*16
NECS = 65 * 64    # 4160: w-major (h<64) ec storage


def r32(x):
    return x.bitcast(F32R)


def ap(tile, part0, nparts, free_off, free_dims):
    """AP over a tile: partitions [part0, part0+nparts), free offset + dims
    (list of [step, count], outer->inner)."""
    pitch = tile.ap[0][0]
    return bass.AP(tile.tensor, tile.offset + part0 * pitch + free_off,
                   [[pitch, nparts]] + [list(d) for d in free_dims])


# ----------------------------------------------------------------------------
# Host-side weight prep
# ----------------------------------------------------------------------------
def prep_core_inputs(inputs, b, g):
    inp = {k: np.ascontiguousarray(np.asarray(v), dtype=np.float32)
           for k, v in inputs.items()}
    heads = list(range(4 * g, 4 * g + 4))

    xp = np.zeros((64, IMGP, IMGP), np.float32)
    xp[:, 1:129, 1:129] = inp["x"][b]
    xp = xp.reshape(64, IMGP * IMGP)

    W1 = np.zeros((2, 2, 64, 128), np.float32)
    for co in range(128):
        W1[:, :, co // 2, co] = inp["w_in"][co, 0] * inp["s_in"][co]
    W1 = W1.reshape(4, 64, 128).transpose(1, 0, 2).reshape(64, 512)
    b1 = inp["b_in"].reshape(128, 1)

    # rcv conv weights. G1 (compact q): cols = [rq 4hx8 | rk | cq | ck].
    # G2 (v, padded): col 32*hi + dd  holds v-channel dd of head hi.
    w_rcv = inp["w_rcv"] * inp["s_rcv"][:, None, None, None]
    qrows = []
    for blk in range(4):           # rq, rk, cq, ck
        for h in heads:
            qrows.extend(range(h * 48 + blk * 8, h * 48 + blk * 8 + 8))
    Wq = w_rcv[qrows]              # [128, 128, 3, 3]
    bq = inp["b_rcv"][qrows].copy()
    scale_mask = np.ones(128, np.float32)
    scale_mask[0:32] = SCALE       # rq
    scale_mask[64:96] = SCALE      # cq
    Wq = Wq * scale_mask[:, None, None, None]
    bq = bq * scale_mask
    Wv = np.zeros((128, 128, 3, 3), np.float32)   # padded v rows
    bv = np.zeros((128, 1), np.float32)
    for hi, h in enumerate(heads):
        for dd in range(16):
            Wv[32 * hi + dd] = w_rcv[h * 48 + 32 + dd]
            bv[32 * hi + dd, 0] = inp["b_rcv"][h * 48 + 32 + dd]
    # lhsT [ci=128, 9 taps, 256 cols (G1 128 | G2 128)]
    Wrcv = np.concatenate(
        [Wq.transpose(1, 2, 3, 0).reshape(128, 9, 128),
         Wv.transpose(1, 2, 3, 0).reshape(128, 9, 128)], axis=2
    ).reshape(128, 9 * 256)
    brcv_g1 = bq.reshape(128, 1)
    brcv_g2 = bv

    # pe conv: input/output both padded to 128 (head hi at rows/cols 32*hi)
    w_pe = inp["w_pe"] * inp["s_pe"][:, None, None, None]
    Wpe = np.zeros((128, 3, 3, 128), np.float32)
    bpe = np.zeros((128, 1), np.float32)
    for hi, h_abs in enumerate(heads):
        for col in range(16):
            co = h_abs * 16 + col
            col_l = 32 * hi + col
            for k in range(2):
                ci_row = 32 * hi + 2 * (col // 2) + k
                Wpe[ci_row, :, :, col_l] = w_pe[co, k]
            bpe[col_l, 0] = inp["b_pe"][co]
    Wpe = Wpe.reshape(128, 9 * 128)

    w_dc = inp["w_dc"]
    g0 = heads[0] * 8
    Wdc = np.zeros((128, 2, 2, 32), np.float32)   # rows = padded A channels
    bdc = np.zeros((32, 1), np.float32)
    for cl in range(32):
        co = g0 + cl
        hi, c = cl // 8, cl % 8
        for k in range(2):
            Wdc[32 * hi + 2 * c + k, :, :, cl] = w_dc[co, k]
        bdc[cl, 0] = inp["b_dc"][co]
    Wdc = Wdc.reshape(128, 4 * 32)

    # final conv, 96-deep (ky folded into contraction): rows (ky, ci32),
    # cols (kx, co64)
    w_out = inp["w_out"] * inp["s_out"][:, None, None, None]   # [64,64,3,3]
    Wout3 = np.zeros((96, 3, 64), np.float32)
    for ky in range(3):
        for ci in range(32):
            for kx in range(3):
                Wout3[ky * 32 + ci, kx, :] = w_out[:, 32 * g + ci, ky, kx]
    Wout3 = Wout3.reshape(96, 192)
    bfin = (inp["b_out"] if g == 0 else np.zeros(64, np.float32)).reshape(64, 1)

    return {
        "xp": xp.astype(BF16NP), "W1": np.ascontiguousarray(W1).astype(BF16NP), "b1": b1,
        "Wrcv": np.ascontiguousarray(Wrcv).astype(BF16NP),
        "brcv_g1": brcv_g1, "brcv_g2": brcv_g2,
        "Wpe": np.ascontiguousarray(Wpe), "bpe": bpe,
        "Wdc": np.ascontiguousarray(Wdc).astype(BF16NP), "bdc": bdc,
        "Wout3": np.ascontiguousarray(Wout3).astype(BF16NP), "bfin": bfin,
        "ident": np.eye(128, dtype=np.float32),
        "ones": np.ones((65, 2), np.float32).astype(BF16NP),
        "zeros": np.zeros((128, PADPIX), np.float32),
    }


# ----------------------------------------------------------------------------
# Device program
# ----------------------------------------------------------------------------
def build_nc():
    nc = bacc.Bacc(None, target_bir_lowering=False)

    dins = {}
    for name, shape, dt_ in [
        ("xp", [64, IMGP * IMGP], BF16), ("W1", [64, 512], BF16),
        ("b1", [128, 1], F32),
        ("Wrcv", [128, 2304], BF16), ("brcv_g1", [128, 1], F32),
        ("brcv_g2", [128, 1], F32),
        ("Wpe", [128, 1152], F32R), ("bpe", [128, 1], F32),
        ("Wdc", [128, 128], BF16), ("bdc", [32, 1], F32),
        ("Wout3", [96, 192], BF16), ("bfin", [64, 1], F32),
        ("ident", [128, 128], F32R),
        ("ones", [65, 2], BF16),
        ("zeros", [128, PADPIX], F32R),
    ]:
        dins[name] = nc.dram_tensor(name, shape, dt_, kind="ExternalInput")
    out_d = nc.dram_tensor("out", [64, IMG, IMG], BF16, kind="ExternalOutput")
    zbf = dins["zeros"].bitcast(BF16)   # [128, 2*PADPIX] of bf16 zeros

    with TileContext(nc) as tc:
        with (
            tc.tile_pool(name="wpool", bufs=1) as wp,
            tc.tile_pool(name="vpool", bufs=1) as vp_,
            tc.tile_pool(name="apool", bufs=1) as ap_,
        ):
            def load(name, shape, dt_=F32):
                t = wp.tile(shape, dt_, tag=name)
                # weights go on the Pool SWDGE queue so the x/W1 loads on
                # the SP/Act HWDGE queues start immediately
                eng = nc.gpsimd if shape[0] * shape[1] > 4096 else nc.sync
                eng.dma_start(out=t[:, :], in_=dins[name][:, :])
                return t

            Wrcv = load("Wrcv", [128, 2304], BF16)
            brg1 = load("brcv_g1", [128, 1])
            brg2 = load("brcv_g2", [128, 1])
            Wpe = load("Wpe", [128, 1152], F32R)
            bpe = load("bpe", [128, 1])
            Wdc = load("Wdc", [128, 128], BF16)
            bdc = load("bdc", [32, 1])
            Wout3 = load("Wout3", [96, 192], BF16)
            bfin = load("bfin", [64, 1])
            ident = load("ident", [128, 128], F32R)
            ones65 = load("ones", [65, 2], BF16)

            v_sb = vp_.tile([128, PADPIX + 2 * HP], F32R, tag="v")  # (h,w) pad
            nc.gpsimd.memset(v_sb[:, :].bitcast(F32), 0.0)
            A_sb = ap_.tile([128, NPIX], BF16, tag="A")      # (w,h)-major
            P_sb = ap_.tile([128, NPIX], BF16, tag="P")      # pe-conv out
            # zero only the pad rows (16-31 of each 32-row head block)
            for hi in range(4):
                nc.gpsimd.dma_start(out=A_sb[32 * hi + 16:32 * hi + 32, :],
                                    in_=zbf[:16, :NPIX])

            with tc.tile_pool(name="qxpool", bufs=1) as qx:
                qQ = qx.tile([128, NPIX + H], BF16, tag="qQ")
                qK = qx.tile([128, NPIX + H], BF16, tag="qK")
                qC1 = qx.tile([128, NPIX + H], BF16, tag="qC1")
                qC2 = qx.tile([128, NPIX + H], BF16, tag="qC2")
                for _t in (qQ, qK, qC1, qC2):
                    nc.gpsimd.memset(_t[:, NPIX:], 0.0)

                with tc.tile_pool(name="ypool", bufs=1) as yp:
                    y_sb = yp.tile([128, PADPIX + 2 * HP + 1], BF16, tag="y")
                    nc.gpsimd.memset(y_sb[:, :].bitcast(F32), 0.0)

                    # ===== S1: in_proj (x loaded in two halves) =====
                    with (
                        tc.tile_pool(name="xpool", bufs=2) as xp_pool,
                        tc.tile_pool(name="ps1", bufs=2, space="PSUM") as ps1,
                    ):
                        W1 = xp_pool.tile([64, 512], BF16, tag="w1")
                        nc.sync.dma_start(out=W1[:, :], in_=dins["W1"][:, :])
                        b1 = xp_pool.tile([128, 1], F32, tag="b1")
                        nc.sync.dma_start(out=b1[:, :], in_=dins["b1"][:, :])

                        chunks = [(0, 7), (7, 7), (14, 7), (21, 7), (28, 4),
                                  (32, 7), (39, 7), (46, 7), (53, 7), (60, 5)]
                        for half in range(2):
                            xt = xp_pool.tile([64, 68 * IMGP], BF16, tag="x")
                            src_off = 0 if half == 0 else 64 * IMGP
                            nc.sync.dma_start(
                                out=xt[:, :33 * IMGP],
                                in_=dins["xp"][:, src_off:src_off + 33 * IMGP])
                            nc.scalar.dma_start(
                                out=xt[:, 33 * IMGP:66 * IMGP],
                                in_=dins["xp"][:, src_off + 33 * IMGP:
                                               src_off + 66 * IMGP])
                            nc.gpsimd.memset(xt[:, 66 * IMGP:].bitcast(F32), 0.0)
                            row0 = 0 if half == 0 else 64
                            for c0, nr in chunks:
                                if (half == 0) != (c0 < 32):
                                    continue
                                pt = ps1.tile([128, 7 * 66], F32, tag="ps1")
                                for t, (ky, kx) in enumerate(
                                        [(0, 0), (0, 1), (1, 0), (1, 1)]):
                                    rhs = ap(xt, 0, 64,
                                             (2 * c0 + ky - row0) * IMGP + kx,
                                             [[2 * IMGP, nr], [2, 66]])
                                    nc.tensor.matmul(
                                        pt[:, :nr * 66],
                                        W1[:, t * 128:(t + 1) * 128],
                                        rhs, start=(t == 0), stop=(t == 3))
                                dst = ap(y_sb, 0, 128, (c0 + 1) * HP + 1,
                                         [[HP, nr], [1, H]])
                                nc.scalar.activation(dst,
                                                     ap(pt, 0, 128, 0,
                                                        [[66, nr], [1, H]]),
                                                     AF.Identity, bias=b1[:, :])

                    # ===== S2: rcv conv (q compact bf16 + v padded f32r) ====
                    with (
                        tc.tile_pool(name="qcpool", bufs=1) as qcp,
                        tc.tile_pool(name="ps2", bufs=2, space="PSUM") as ps2,
                    ):
                        q_sb = qcp.tile([128, NPIX], BF16, tag="qc")
                        for c0 in range(0, H, 7):
                            nr = min(7, H - c0)
                            pt = ps2.tile([128, 7 * 66], F32, tag="ps2")
                            for t in range(9):
                                ky, kx = t // 3, t % 3
                                rhs = ap(y_sb, 0, 128, (c0 + ky) * HP + kx,
                                         [[HP, nr], [1, 66]])
                                nc.tensor.matmul(
                                    pt[:, :nr * 66],
                                    Wrcv[:, t * 256:t * 256 + 128],
                                    rhs, start=(t == 0), stop=(t == 8))
                            nc.scalar.activation(q_sb[:, c0 * H:(c0 + nr) * H],
                                                 ap(pt, 0, 128, 0,
                                                    [[66, nr], [1, H]]),
                                                 AF.Identity, bias=brg1[:, :])
                            pt2 = ps2.tile([128, 7 * 66], F32, tag="ps2")
                            for t in range(9):
                                ky, kx = t // 3, t % 3
                                rhs = ap(y_sb, 0, 128, (c0 + ky) * HP + kx,
                                         [[HP, nr], [1, 66]])
                                nc.tensor.matmul(
                                    pt2[:, :nr * 66],
                                    Wrcv[:, t * 256 + 128:t * 256 + 256],
                                    rhs, start=(t == 0), stop=(t == 8))
                            dstv = ap(v_sb, 0, 128, (c0 + 1) * HP + 1,
                                      [[HP, nr], [1, H]])
                            nc.scalar.activation(dstv,
                                                 ap(pt2, 0, 128, 0,
                                                    [[66, nr], [1, H]]),
                                                 AF.Identity, bias=brg2[:, :])
                        # reshuffle q -> 32-aligned padded tensors (sbuf
                        # dma, spread across DGE queues to parallelize issue)
                        qeng = [nc.sync, nc.scalar]
                        for hi in range(4):
                            for blk, dstq in enumerate([qQ, qK, qC1, qC2]):
                                qeng[(hi * 4 + blk) % 2].dma_start(
                                    out=ap(dstq, 32 * hi, 8, 0, [[1, NPIX]]),
                                    in_=q_sb[blk * 32 + 8 * hi:
                                             blk * 32 + 8 * hi + 8, :])

                # ===== S3-S6: attention, software-pipelined per head =====
                # Per-head prologue (scores/exp/Z/iz/V-permute) is emitted as
                # generator steps interleaved into the PREVIOUS head's chunk
                # loop, so PE-heavy score work overlaps DVE/Pool-heavy chunks.
                with (
                    tc.tile_pool(name="hpool", bufs=2) as hp,
                    tc.tile_pool(name="mpool", bufs=4) as mp,
                    tc.tile_pool(name="up2pool", bufs=3) as up2,
                    tc.tile_pool(name="mhpool", bufs=3) as mhp,
                    tc.tile_pool(name="m2pool", bufs=2) as m2p,
                    tc.tile_pool(name="tpool", bufs=8) as tp,
                    tc.tile_pool(name="scps", bufs=1, space="PSUM") as scps,
                    tc.tile_pool(name="ups", bufs=2, space="PSUM") as ups,
                    tc.tile_pool(name="pttps", bufs=1, space="PSUM") as pttp,
                ):
                    _padded_heads = set()
                    _padded_m = set()

                    def alloc_head():
                        t = {}
                        for nm, shape, dt_ in [
                            ("er", [65, 66 * H], BF16),      # (w, i) 66-stride
                            ("er2", [128, 66 * H], BF16),
                            ("ertail", [65, 65], BF16),
                            ("ecs", [65, NECS], BF16),
                            ("ectail", [65, 65], BF16),
                            ("zf1", [65, 34 * H], BF16),     # Zr fold1 (w,34)
                            ("zravg", [65, 65], F32),
                            ("zrc", [128, 34], F32),
                            ("zcc", [128, 34], F32),
                            ("iz2", [128, 34], F32),
                            ("vpt", [65, ID16], BF16),
                        ]:
                            tl = hp.tile(shape, dt_, tag=nm)
                            t[nm] = tl
                        # zero pad columns read by the fold chains (col 65
                        # of each er 66-block; col 33 of zf1; col 17 of zf2)
                        if t["er"].offset not in _padded_heads:
                            _padded_heads.add(t["er"].offset)
                            nc.gpsimd.memset(
                                ap(t["er"], 0, 65, 65, [[66, H], [1, 1]]), 0.0)
                            nc.gpsimd.memset(
                                ap(t["zf1"], 0, 65, 33, [[34, H], [1, 1]]), 0.0)
                        return t

                    def prologue_steps(hi, t):
                        """Generator: emits one instruction group per next()."""
                        tpos = (32 * hi, 0)
                        er, ecs, ectail = t["er"], t["ecs"], t["ectail"]
                        # V-permute first (only needs v_sb)
                        for i0 in range(0, H, 32):
                            ni = min(32, H - i0)
                            ptv = scps.tile([128, 512], F32, tag="sc")
                            for k in range(ni):
                                i = i0 + k
                                src = ap(v_sb, 32 * hi, 16,
                                         (i + 1) * HP + 1, [[1, H]])
                                idn = ap(ident, 32 * hi, 16, 32 * hi,
                                         [[1, 16]])
                                nc.tensor.transpose(
                                    r32(ap(ptv, 0, 65, k * 16, [[1, 16]])),
                                    src, idn, tile_position=tpos)
                            nc.scalar.activation(
                                ap(t["vpt"], 0, 65, i0, [[1, ni], [H, 16]]),
                                ptv[:65, :ni * 16], AF.Copy)
                            yield
                        # r scores: per w -> psum [h, i]; exp -> er (bf16,
                        # 66-stride per w so the Zr fold chain needs no tail)
                        for w0 in range(0, H, 7):
                            nw = min(7, H - w0)
                            pt = scps_cur[0].tile([128, 512], F32, tag="sc")
                            for k in range(nw):
                                w = w0 + k
                                nc.tensor.matmul(
                                    pt[:65, k * 66:k * 66 + 66],
                                    ap(qK, 32 * hi, 8, w, [[H, H]]),
                                    ap(qQ, 32 * hi, 8, w, [[H, 66]]),
                                    start=True, stop=True,
                                    tile_position=tpos)
                            nc.scalar.activation(
                                ap(er, 0, 65, w0 * 66, [[66, nw], [1, H]]),
                                ap(pt, 0, 65, 0, [[66, nw], [1, H]]),
                                AF.Exp)
                            yield
                        # er2 (pixel-partitioned) + ertail
                        nc.scalar.dma_start(out=t["er2"][0:64, :],
                                            in_=er[0:64, :])
                        yield
                        nc.scalar.dma_start(out=t["er2"][64:128,
                                                         :66 * H - 66],
                                            in_=er[0:64, 66:])
                        yield
                        nc.scalar.dma_start(out=t["ertail"][:, :],
                                            in_=ap(er, 64, 1, 0,
                                                   [[66, H], [1, H]]))
                        yield
                        # Zr: fold (w,66)->(w,34) on Pool in 4 pieces, then
                        # DVE reduce-34 -> zravg[h,w]
                        zf1 = t["zf1"]
                        for wz0 in range(0, H, 17):
                            nwz = min(17, H - wz0)
                            pool_fold(
                                ap(zf1, 0, 65, wz0 * 34, [[34, nwz], [1, 33]]),
                                ap(er, 0, 65, wz0 * 66, [[66, nwz], [1, 33]]),
                                ap(er, 0, 65, wz0 * 66 + 33,
                                   [[66, nwz], [1, 33]]))
                            yield
                        nc.vector.tensor_reduce(
                            t["zravg"][:, :],
                            ap(zf1, 0, 65, 0, [[34, H], [1, 34]]),
                            AX.X, AluOpType.add)
                        yield
                        # c scores: per h -> psum [j, w]; exp -> ecs (f32r)
                        for h0 in range(0, H, 7):
                            nh = min(7, H - h0)
                            pt = scps_cur[0].tile([128, 512], F32, tag="sc")
                            for k in range(nh):
                                h = h0 + k
                                nc.tensor.matmul(
                                    pt[:65, k * 66:k * 66 + 66],
                                    ap(qC1, 32 * hi, 8, h * H, [[1, H]]),
                                    ap(qC2, 32 * hi, 8, h * H, [[1, 66]]),
                                    start=True, stop=True,
                                    tile_position=tpos)
                            nhs = min(nh, 64 - h0)
                            nc.scalar.activation(
                                ap(ecs, 0, 65, h0, [[1, nhs], [64, H]]),
                                ap(pt, 0, 65, 0, [[66, nhs], [1, H]]),
                                AF.Exp)
                            if h0 + nh == 65:
                                nc.scalar.activation(
                                    ectail[:, :],
                                    ap(pt, 0, 65, (nh - 1) * 66, [[1, H]]),
                                    AF.Exp)
                            yield
                        # Zc per chunk (chunk-partitioned ones-matmuls)
                        zct = scps_cur[0].tile([128, 512], F32, tag="sc")
                        for wb0 in range(0, 32, 16):
                            for wb in range(wb0, wb0 + 16):
                                nc.tensor.matmul(
                                    ap(zct, 0, 128, 2 * wb, [[1, 2]]),
                                    ap(ecs, 0, 65, wb * 128, [[1, 128]]),
                                    ones65[:, :], start=True, stop=True)
                            yield
                        nc.tensor.matmul(
                            ap(zct, 0, 64, 64, [[1, 2]]),
                            ap(ecs, 0, 65, 64 * 64, [[1, 64]]),
                            ones65[:, :], start=True, stop=True)
                        nc.tensor.matmul(
                            ap(zct, 0, 65, 66, [[1, 2]]),
                            ap(ectail, 0, 65, 0, [[1, H]]),
                            ones65[:, :], start=True, stop=True)
                        zcc = t["zcc"]
                        nc.scalar.activation(zcc[0:64, :],
                                             ap(zct, 0, 64, 0, [[2, 34]]),
                                             AF.Copy)
                        nc.scalar.activation(zcc[64:128, 0:32],
                                             ap(zct, 64, 64, 0, [[2, 32]]),
                                             AF.Copy)
                        nc.scalar.activation(zcc[64:65, 33:34],
                                             ap(zct, 64, 1, 66, [[1, 1]]),
                                             AF.Copy)
                        yield
                        # zr chunk columns + iz scalars
                        zravg, zrc = t["zravg"], t["zrc"]
                        nc.vector.tensor_copy(
                            ap(zrc, 0, 64, 0, [[1, 32]]),
                            ap(zravg, 0, 64, 0, [[2, 32]]))
                        nc.vector.tensor_copy(
                            ap(zrc, 64, 64, 0, [[1, 32]]),
                            ap(zravg, 0, 64, 1, [[2, 32]]))
                        nc.vector.tensor_copy(zrc[0:64, 32:33],
                                              zravg[0:64, 64:65])
                        nc.scalar.dma_start(out=zrc[0:65, 33:34],
                                            in_=ap(zravg, 64, 1, 0, [[1, H]]))
                        iz2 = t["iz2"]
                        nc.vector.tensor_tensor(out=iz2[:, :], in0=zrc[:, :],
                                                in1=zcc[:, :],
                                                op=AluOpType.mult)
                        nc.vector.reciprocal(iz2[:, :], iz2[:, :])
                        yield

                    def pe_p_steps():
                        """S7 pe-conv into standalone P (only needs v_sb)."""
                        for w0 in range(0, H, 7):
                            nw = min(7, H - w0)
                            pt = scps_cur[0].tile([128, 512], F32, tag="sc")
                            for tt in range(9):
                                ky, kx = tt // 3, tt % 3
                                rhs = ap(v_sb, 0, 128, ky * HP + kx + w0,
                                         [[1, nw], [HP, 66]])
                                nc.tensor.matmul(
                                    pt[:, :nw * 66],
                                    Wpe[:, tt * 128:tt * 128 + 128],
                                    rhs, start=(tt == 0), stop=(tt == 8))
                            nc.scalar.activation(
                                P_sb[:, w0 * H:(w0 + nw) * H],
                                ap(pt, 0, 128, 0, [[66, nw], [1, H]]),
                                AF.Identity, bias=bpe[:, :])
                            yield

                    state = {"ptt": None, "off": 0}
                    pend = []

                    def pool_fold(out, in0, in1):
                        nc.gpsimd.tensor_tensor(out=out, in0=in0, in1=in1,
                                                op=AluOpType.add)

                    def do_chunk(t, idx, lhsT_ap, er_ap, izcol, M, dst):
                        ut = ups.tile([128, ID16], F32, tag="ut")
                        for n0 in (0, 512, 1024):
                            nn = min(512, ID16 - n0)
                            nc.tensor.matmul(ut[:M, n0:n0 + nn],
                                             lhsT_ap,
                                             t["vpt"][:, n0:n0 + nn],
                                             start=True, stop=True)
                        # m is (d, i66)-strided bf16; col 65 of each d-block
                        # is pre-zeroed so the fold chain needs no tail ops
                        # E-chunks (DVE stt from PSUM) 1 in 4; fold2 on DVE
                        ph = 2 if idx % 4 == 0 else 0
                        m = mp.tile([128, 1066], BF16, tag="m")
                        if m.offset not in _padded_m:
                            _padded_m.add(m.offset)
                            nc.gpsimd.memset(
                                ap(m, 0, 128, 65, [[66, 16], [1, 1]]), 0.0)
                        if ph == 2:
                            # DVE: (ut * iz) * er straight out of PSUM (1x)
                            nc.vector.scalar_tensor_tensor(
                                out=ap(m, 0, M, 0, [[66, 16], [1, 65]]),
                                in0=ap(ut, 0, M, 0, [[65, 16], [1, 65]]),
                                scalar=t["iz2"][:M, izcol:izcol + 1],
                                in1=er_ap,
                                op0=AluOpType.mult, op1=AluOpType.mult)
                        else:
                            # Act drains PSUM to bf16 applying iz via scale;
                            # DVE multiplies by raw er at 2x
                            utb = up2.tile([128, ID16], BF16, tag="utb")
                            nc.scalar.activation(
                                ap(utb, 0, M, 0, [[65, 16], [1, 65]]),
                                ap(ut, 0, M, 0, [[65, 16], [1, 65]]),
                                AF.Identity,
                                scale=t["iz2"][:M, izcol:izcol + 1])
                            nc.vector.tensor_tensor(
                                out=ap(m, 0, M, 0, [[66, 16], [1, 65]]),
                                in0=ap(utb, 0, M, 0, [[65, 16], [1, 65]]),
                                in1=er_ap, op=AluOpType.mult)
                        # Pool fold 66 -> 34 via stt-add (col 33 pre-zeroed)
                        mh = mhp.tile([128, 544], BF16, tag="mh")
                        if mh.offset not in _padded_m:
                            _padded_m.add(mh.offset)
                            nc.gpsimd.memset(
                                ap(mh, 0, 128, 33, [[34, 16], [1, 1]]), 0.0)
                        pool_fold(ap(mh, 0, M, 0, [[34, 16], [1, 33]]),
                                  ap(m, 0, M, 0, [[66, 16], [1, 33]]),
                                  ap(m, 0, M, 33, [[66, 16], [1, 33]]))
                        # fold2+reduce deferred one chunk so DVE never
                        # stalls waiting on this chunk's Pool fold1
                        pend2.append((mh, M, dst))
                        if len(pend2) > 1:
                            do_tail(*pend2.pop(0))

                    def do_tail(mh, M, dst):
                        m2 = m2p.tile([128, 272], BF16, tag="m2")
                        nc.vector.tensor_tensor(
                            out=ap(m2, 0, M, 0, [[17, 16], [1, 17]]),
                            in0=ap(mh, 0, M, 0, [[34, 16], [1, 17]]),
                            in1=ap(mh, 0, M, 17, [[34, 16], [1, 17]]),
                            op=AluOpType.add)
                        at = tp.tile([128, 16], F32, tag="at")
                        nc.vector.tensor_reduce(
                            at[:M, :], ap(m2, 0, M, 0, [[17, 16], [1, 17]]),
                            AX.X, AluOpType.add)
                        pend.append((at, M, dst))

                    def emit_transpose(at, M):
                        if state["ptt"] is None:
                            pttt = pttp.tile([128, 512], F32, tag="ptt")
                            state["ptt"] = pttt
                            state["off"] = 0
                        nc.tensor.transpose(
                            ap(state["ptt"], 0, 16, state["off"], [[1, M]]),
                            at[:M, :], ident[:M, :M].bitcast(F32))
                        state["off"] += M

                    def drain_pend(keep):
                        while len(pend) > keep:
                            at, M, dst = pend.pop(0)
                            emit_transpose(at, M)
                            if dst is not None:
                                nc.scalar.activation(
                                    dst,
                                    ap(state["ptt"], 0, 16, 0,
                                       [[1, state["off"]]]),
                                    AF.Copy)
                                state["ptt"] = None

                    tiles = alloc_head()
                    for _ in prologue_steps(0, tiles):
                        pass
                    for hi in range(4):
                        A0 = 32 * hi
                        t = tiles
                        if hi < 3:
                            tiles = alloc_head()
                            nxt = prologue_steps(hi + 1, tiles)
                        else:
                            nxt = pe_p_steps()
                        er, ertail = t["er"], t["ertail"]
                        for wb in range(32):
                            dst = (ap(A_sb, A0, 16, 2 * (wb - 3) * H,
                                      [[H, 8], [1, 64]])
                                   if wb % 4 == 3 else None)
                            do_chunk(t, wb,
                                     ap(t["ecs"], 0, 65, wb * 128, [[1, 128]]),
                                     ap(t["er2"], 0, 128, 2 * wb * 66,
                                        [[0, 16], [1, H]]),
                                     wb, 128, dst)
                            drain_pend(4)
                            next(nxt, None)
                        do_chunk(t, 32,
                                 ap(t["ecs"], 0, 65, 64 * 64, [[1, 64]]),
                                 ap(t["er2"], 0, 64, 64 * 66,
                                    [[0, 16], [1, H]]),
                                 32, 64, ap(A_sb, A0, 16, 64 * H, [[1, 64]]))
                        drain_pend(4)
                        next(nxt, None)
                        do_chunk(t, 33,
                                 ap(t["ectail"], 0, 65, 0, [[1, H]]),
                                 ap(ertail, 0, 65, 0, [[0, 16], [1, H]]),
                                 33, 65, ap(A_sb, A0, 16, 64, [[H, H]]))
                        drain_pend(4)
                        for _ in nxt:
                            pass
                    drain_pend(0)

            # ===== S8: dconv(A) + dconv(P) -> dc3 rows 0:32 =====
            with (
                tc.tile_pool(name="dcpool", bufs=1) as dcp,
                tc.tile_pool(name="ps8", bufs=2, space="PSUM") as ps8,
            ):
                dc3 = dcp.tile([96, IMGP * IMGP], BF16, tag="dc3")
                # zero borders: block b holds dcpad rows shifted by b, so
                # block0 rows {0,129}, block1 rows {128,129-ish}, block2
                # rows {127,128}; plus the 1-px column strips everywhere.
                nc.sync.dma_start(out=ap(dc3, 0, 32, 0, [[1, IMGP]]),
                                  in_=zbf[:32, :IMGP])
                nc.sync.dma_start(
                    out=ap(dc3, 0, 32, 129 * IMGP, [[1, IMGP]]),
                    in_=zbf[:32, :IMGP])
                nc.sync.dma_start(
                    out=ap(dc3, 32, 32, 128 * IMGP, [[1, 2 * IMGP]]),
                    in_=zbf[:32, :2 * IMGP])
                nc.sync.dma_start(
                    out=ap(dc3, 64, 32, 127 * IMGP, [[1, 3 * IMGP]]),
                    in_=zbf[:32, :3 * IMGP])
                for blk in range(3):
                    nc.sync.dma_start(
                        out=ap(dc3, 32 * blk, 32, IMGP, [[IMGP, 128], [1, 1]]),
                        in_=zbf[:32, :128])
                    nc.sync.dma_start(
                        out=ap(dc3, 32 * blk, 32, IMGP + 129,
                               [[IMGP, 128], [1, 1]]),
                        in_=zbf[:32, :128])
                # a0-outer so dc rows complete in ascending order; the
                # row-shifted copies for the 96-deep final conv are issued
                # piecewise so S9 can pipeline behind S8.
                shift_done = 0

                def dc3_shift_upto(row):
                    nonlocal shift_done
                    lo = shift_done
                    if row <= lo:
                        return
                    nc.sync.dma_start(
                        out=dc3[32:64, lo * IMGP:row * IMGP],
                        in_=dc3[0:32, (lo + 1) * IMGP:(row + 1) * IMGP])
                    nc.scalar.dma_start(
                        out=dc3[64:96, lo * IMGP:row * IMGP],
                        in_=dc3[0:32, (lo + 2) * IMGP:(row + 2) * IMGP])
                    shift_done = row

                for a0 in range(0, 64, 8):
                    for pr in range(2):
                        for ps in range(2):
                            pt = ps8.tile([32, 512], F32, tag="dcps")
                            w0 = (pr * 2 + ps) * 32
                            nc.tensor.matmul(
                                pt[:, :], Wdc[:, w0:w0 + 32],
                                ap(A_sb, 0, 128, ps * H + pr + a0,
                                   [[1, 8], [H, 64]]),
                                start=True, stop=False)
                            nc.tensor.matmul(
                                pt[:, :], Wdc[:, w0:w0 + 32],
                                ap(P_sb, 0, 128, ps * H + pr + a0,
                                   [[1, 8], [H, 64]]),
                                start=False, stop=True)
                            dst = ap(dc3, 0, 32,
                                     (2 * a0 + pr + 1) * IMGP + ps + 1,
                                     [[2 * IMGP, 8], [2, 64]])
                            if (pr * 2 + ps) % 2 == 0:
                                nc.scalar.activation(dst, pt[:, :],
                                                     AF.Identity,
                                                     bias=bdc[:, :])
                            else:
                                nc.vector.scalar_tensor_tensor(
                                    out=dst, in0=pt[:, :], scalar=1.0,
                                    in1=ap(bdc, 0, 32, 0, [[0, 8], [0, 64]]),
                                    op0=AluOpType.mult, op1=AluOpType.add)
                    if a0 in (24, 40, 56):
                        # rows complete up to 2*a0+16 after this block
                        dc3_shift_upto(2 * a0 + 14)
                nc.sync.dma_start(
                    out=dc3[32:64, shift_done * IMGP:IMGP * IMGP - IMGP],
                    in_=dc3[0:32, (shift_done + 1) * IMGP:])
                nc.scalar.dma_start(
                    out=dc3[64:96, shift_done * IMGP:IMGP * IMGP - 2 * IMGP],
                    in_=dc3[0:32, (shift_done + 2) * IMGP:IMGP * IMGP])

                # ===== S9: final conv partial, 96-deep =====
                with (
                    tc.tile_pool(name="opool", bufs=2) as op_,
                    tc.tile_pool(name="ps9", bufs=2, space="PSUM") as ps9,
                ):
                    ost = None
                    for r0 in range(0, IMG, 4):
                        pt = ps9.tile([64, 512], F32, tag="o")
                        for kx in range(3):
                            rhs = ap(dc3, 0, 96, r0 * IMGP + kx,
                                     [[IMGP, 4], [1, IMG]])
                            nc.tensor.matmul(pt[:, :],
                                             Wout3[:, kx * 64:kx * 64 + 64],
                                             rhs, start=(kx == 0),
                                             stop=(kx == 2))
                        if r0 % 16 == 0:
                            ost = op_.tile([64, 2048], BF16, tag="ost")
                        sl = (r0 % 16) // 4 * 512
                        if (r0 // 4) % 2 == 0:
                            nc.scalar.activation(ost[:, sl:sl + 512], pt[:, :],
                                                 AF.Identity, bias=bfin[:, :])
                        else:
                            nc.vector.scalar_tensor_tensor(
                                out=ost[:, sl:sl + 512], in0=pt[:, :],
                                scalar=1.0,
                                in1=ap(bfin, 0, 64, 0, [[0, 512]]),
                                op0=AluOpType.mult, op1=AluOpType.add)
                        if r0 % 16 == 12:
                            oeng = nc.sync if r0 % 32 == 12 else nc.scalar
                            oeng.dma_start(out=out_d[:, r0 - 12:r0 + 4, :],
                                           in_=ost[:, :])

    nc.compile()
    return nc


_NC_CACHE = None


def kernel(**inputs):
    global _NC_CACHE
    if _NC_CACHE is None:
        _NC_CACHE = build_nc()
    nc = _NC_CACHE
    in_maps = [prep_core_inputs(inputs, c // 2, c % 2) for c in range(8)]
    res = run_bass_kernel_spmd(nc, in_maps, list(range(8)))
    out = np.zeros((4, 64, IMG, IMG), np.float32)
    for b in range(4):
        out[b] = (res.results[2 * b]["out"].astype(np.float32) +
                  res.results[2 * b + 1]["out"].astype(np.float32))
    return out



# revision 39
# speedup vs baseline: 1.1843x; 1.0794x over previous
"""Trainium2 Bass kernel for nn_MatrixAttention (sparse_attention).

Sharding: 8 cores = (batch b in 0..3) x (head-group g in 0..1, 4 heads each).
Each core: in_proj -> rcv conv (its 192 ch) -> row/col attention (4 heads)
-> pe conv -> grouped deconv (its 32 dc ch) -> partial final 3x3 conv over
all 64 output channels from its 32 dc channels. Host gather sums the pair
partials (input-dim-sharded conv => reduce-gather) and stacks batches.

Perf structure (TimelineSim-tuned):
- q/ecs/vpt/scores all bf16 (f32r matmuls with <256-col outputs pay a 4x
  cycle penalty; bf16 is 1 cycle/row and halves SBUF).
- S2 split: G1 (q) chunks first, then one merged q-reshuffle (sync+gpsimd
  DGE queues; the ~630ns/DMA descriptor-gen serializes on HWDGE, so it
  must never sit on the Act/SP queues mid-pipeline), then G2 (v) chunks
  with the head-0 prologue generator interleaved (scores/exp/Zc/Zr/iz
  hide behind G2's PE work; V-permute last since it needs v).
- Combine chunks (128 px): PE matmul (ec^T V, 65-deep, 1040 cols) ->
  D-chunks: Act drain (iz scale) to bf16 + DVE tensor_tensor x er (2x) |
  E-chunks (1 in 4): DVE stt from PSUM (1x) -> Pool fold 65->33 ->
  DVE reduce-33 -> PE transpose -> Act flush into A.
  The fold2/reduce of chunk N runs at the TOP of chunk N+2 (pend2) and
  transposes/flushes are deferred 4 chunks (pend): in-order engine queues
  otherwise serialize the whole chain per chunk.
- Zr via Pool fold (w,66-stride er, pad col zeroed per head) + DVE
  reduce-33; Zc via ones-matmuls; iz=1/(Zr*Zc) as drain scale/stt scalar.
- S8/S9 drains alternate Act / DVE-stt(+bias broadcast); output stores
  batched 4 row-groups per DMA (HWDGE descriptor-gen is the tail limit).

Self-contained: hardcodes all shapes; no sibling imports.
"""
import sys
import numpy as np

sys.path.insert(0, "/opt/trn_rl_repo")

import ml_dtypes                        # noqa: E402
import concourse.bass as bass           # noqa: E402
import concourse.bacc as bacc           # noqa: E402
import concourse.mybir as mybir         # noqa: E402
from concourse.tile import TileContext  # noqa: E402
from concourse.bass_utils import run_bass_kernel_spmd  # noqa: E402
from concourse.alu_op_type import AluOpType  # noqa: E402

F32 = mybir.dt.float32
F32R = mybir.dt.float32r
BF16 = mybir.dt.bfloat16
AF = mybir.ActivationFunctionType
AX = mybir.AxisListType
BF16NP = ml_dtypes.bfloat16

NH, KD, HD = 8, 8, 16
SCALE = KD ** -0.5
H = 65            # spatial after in_proj
HP = 67           # padded
NPIX = H * H      # 4225
PADPIX = HP * HP  # 4489
IMG = 128
IMGP = 130
ID16 = 1040       # (i,d) = 65*16
NECS = 65 * 64    # 4160: w-major (h<64) ec storage


def r32(x):
    return x.bitcast(F32R)


def ap(tile, part0, nparts, free_off, free_dims):
    """AP over a tile: partitions [part0, part0+nparts), free offset + dims
    (list of [step, count], outer->inner)."""
    pitch = tile.ap[0][0]
    return bass.AP(tile.tensor, tile.offset + part0 * pitch + free_off,
                   [[pitch, nparts]] + [list(d) for d in free_dims])


# ----------------------------------------------------------------------------
# Host-side weight prep
# ----------------------------------------------------------------------------
def prep_core_inputs(inputs, b, g):
    inp = {k: np.ascontiguousarray(np.asarray(v), dtype=np.float32)
           for k, v in inputs.items()}
    heads = list(range(4 * g, 4 * g + 4))

    xp = np.zeros((64, IMGP, IMGP), np.float32)
    xp[:, 1:129, 1:129] = inp["x"][b]
    xp = xp.reshape(64, IMGP * IMGP)

    W1 = np.zeros((2, 2, 64, 128), np.float32)
    for co in range(128):
        W1[:, :, co // 2, co] = inp["w_in"][co, 0] * inp["s_in"][co]
    W1 = W1.reshape(4, 64, 128).transpose(1, 0, 2).reshape(64, 512)
    b1 = inp["b_in"].reshape(128, 1)

    # rcv conv weights. G1 (compact q): cols = [rq 4hx8 | rk | cq | ck].
    # G2 (v, padded): col 32*hi + dd  holds v-channel dd of head hi.
    w_rcv = inp["w_rcv"] * inp["s_rcv"][:, None, None, None]
    qrows = []
    for blk in range(4):           # rq, rk, cq, ck
        for h in heads:
            qrows.extend(range(h * 48 + blk * 8, h * 48 + blk * 8 + 8))
    Wq = w_rcv[qrows]              # [128, 128, 3, 3]
    bq = inp["b_rcv"][qrows].copy()
    scale_mask = np.ones(128, np.float32)
    scale_mask[0:32] = SCALE       # rq
    scale_mask[64:96] = SCALE      # cq
    Wq = Wq * scale_mask[:, None, None, None]
    bq = bq * scale_mask
    Wv = np.zeros((128, 128, 3, 3), np.float32)   # padded v rows
    bv = np.zeros((128, 1), np.float32)
    for hi, h in enumerate(heads):
        for dd in range(16):
            Wv[32 * hi + dd] = w_rcv[h * 48 + 32 + dd]
            bv[32 * hi + dd, 0] = inp["b_rcv"][h * 48 + 32 + dd]
    # lhsT [ci=128, 9 taps, 256 cols (G1 128 | G2 128)]
    Wrcv = np.concatenate(
        [Wq.transpose(1, 2, 3, 0).reshape(128, 9, 128),
         Wv.transpose(1, 2, 3, 0).reshape(128, 9, 128)], axis=2
    ).reshape(128, 9 * 256)
    brcv_g1 = bq.reshape(128, 1)
    brcv_g2 = bv

    # pe conv: input/output both padded to 128 (head hi at rows/cols 32*hi)
    w_pe = inp["w_pe"] * inp["s_pe"][:, None, None, None]
    Wpe = np.zeros((128, 3, 3, 64), np.float32)
    bpe = np.zeros((64, 1), np.float32)
    for hi, h_abs in enumerate(heads):
        for col in range(16):
            co = h_abs * 16 + col
            col_l = 16 * hi + col
            for k in range(2):
                ci_row = 32 * hi + 2 * (col // 2) + k
                Wpe[ci_row, :, :, col_l] = w_pe[co, k]
            bpe[col_l, 0] = inp["b_pe"][co]
    Wpe = Wpe.reshape(128, 9 * 64)

    w_dc = inp["w_dc"]
    g0 = heads[0] * 8
    # rows: A-compact channels 0:64, P-compact channels 64:128 (dconv of
    # A+P done as one 128-deep matmul over the merged AP tile)
    Wdc = np.zeros((128, 2, 2, 32), np.float32)
    bdc = np.zeros((32, 1), np.float32)
    for cl in range(32):
        co = g0 + cl
        hi, c = cl // 8, cl % 8
        for k in range(2):
            Wdc[16 * hi + 2 * c + k, :, :, cl] = w_dc[co, k]
            Wdc[64 + 16 * hi + 2 * c + k, :, :, cl] = w_dc[co, k]
        bdc[cl, 0] = inp["b_dc"][co]
    Wdc = Wdc.reshape(128, 4 * 32)

    # final conv, 96-deep (ky folded into contraction): rows (ky, ci32),
    # cols (kx, co64)
    w_out = inp["w_out"] * inp["s_out"][:, None, None, None]   # [64,64,3,3]
    Wout3 = np.zeros((96, 3, 64), np.float32)
    for ky in range(3):
        for ci in range(32):
            for kx in range(3):
                Wout3[ky * 32 + ci, kx, :] = w_out[:, 32 * g + ci, ky, kx]
    Wout3 = Wout3.reshape(96, 192)
    bfin = (inp["b_out"] if g == 0 else np.zeros(64, np.float32)).reshape(64, 1)

    return {
        "xp": xp.astype(BF16NP), "W1": np.ascontiguousarray(W1).astype(BF16NP), "b1": b1,
        "Wrcv": np.ascontiguousarray(Wrcv).astype(BF16NP),
        "brcv_g1": brcv_g1, "brcv_g2": brcv_g2,
        "Wpe": np.ascontiguousarray(Wpe), "bpe": bpe,
        "Wdc": np.ascontiguousarray(Wdc).astype(BF16NP), "bdc": bdc,
        "Wout3": np.ascontiguousarray(Wout3).astype(BF16NP), "bfin": bfin,
        "ident": np.eye(128, dtype=np.float32),
        "ones": np.ones((65, 2), np.float32).astype(BF16NP),
        "zeros": np.zeros((128, PADPIX), np.float32),
    }


# ----------------------------------------------------------------------------
# Device program
# ----------------------------------------------------------------------------
def build_nc():
    nc = bacc.Bacc(None, target_bir_lowering=False)

    dins = {}
    for name, shape, dt_ in [
        ("xp", [64, IMGP * IMGP], BF16), ("W1", [64, 512], BF16),
        ("b1", [128, 1], F32),
        ("Wrcv", [128, 2304], BF16), ("brcv_g1", [128, 1], F32),
        ("brcv_g2", [128, 1], F32),
        ("Wpe", [128, 576], F32R), ("bpe", [64, 1], F32),
        ("Wdc", [128, 128], BF16), ("bdc", [32, 1], F32),
        ("Wout3", [96, 192], BF16), ("bfin", [64, 1], F32),
        ("ident", [128, 128], F32R),
        ("ones", [65, 2], BF16),
        ("zeros", [128, PADPIX], F32R),
    ]:
        dins[name] = nc.dram_tensor(name, shape, dt_, kind="ExternalInput")
    out_d = nc.dram_tensor("out", [64, IMG, IMG], BF16, kind="ExternalOutput")
    zbf = dins["zeros"].bitcast(BF16)   # [128, 2*PADPIX] of bf16 zeros

    with TileContext(nc) as tc:
        with (
            tc.tile_pool(name="wpool", bufs=1) as wp,
            tc.tile_pool(name="vpool", bufs=1) as vp_,
            tc.tile_pool(name="apool", bufs=1) as ap_,
        ):
            def load(name, shape, dt_=F32):
                t = wp.tile(shape, dt_, tag=name)
                # weights go on the Pool SWDGE queue so the x/W1 loads on
                # the SP/Act HWDGE queues start immediately
                eng = nc.gpsimd if shape[0] * shape[1] > 4096 else nc.sync
                eng.dma_start(out=t[:, :], in_=dins[name][:, :])
                return t

            Wrcv = load("Wrcv", [128, 2304], BF16)
            brg1 = load("brcv_g1", [128, 1])
            brg2 = load("brcv_g2", [128, 1])
            Wpe = load("Wpe", [128, 576], F32R)
            bpe = load("bpe", [64, 1])
            Wdc = load("Wdc", [128, 128], BF16)
            bdc = load("bdc", [32, 1])
            Wout3 = load("Wout3", [96, 192], BF16)
            bfin = load("bfin", [64, 1])
            ident = load("ident", [128, 128], F32R)
            ones65 = load("ones", [65, 2], BF16)

            v_sb = vp_.tile([128, PADPIX + 2 * HP], F32R, tag="v")  # (h,w) pad
            nc.gpsimd.memset(v_sb[:, :].bitcast(F32), 0.0)
            # merged tile: rows 0:64 = attention A (16 per head, compact),
            # rows 64:128 = pe-conv P (compact); no pad rows
            A_sb = ap_.tile([128, NPIX], BF16, tag="A")      # (w,h)-major

            with tc.tile_pool(name="qxpool", bufs=1) as qx:
                qQ = qx.tile([128, NPIX + H], BF16, tag="qQ")
                qK = qx.tile([128, NPIX + H], BF16, tag="qK")
                qC1 = qx.tile([128, NPIX + H], BF16, tag="qC1")
                qC2 = qx.tile([128, NPIX + H], BF16, tag="qC2")
                for _t in (qQ, qK, qC1, qC2):
                    nc.gpsimd.memset(_t[:, NPIX:], 0.0)

                with tc.tile_pool(name="ypool", bufs=1) as yp:
                    y_sb = yp.tile([128, PADPIX + 2 * HP + 1], BF16, tag="y")
                    nc.gpsimd.memset(y_sb[:, :].bitcast(F32), 0.0)

                    # ===== S1: in_proj (x loaded in two halves) =====
                    with (
                        tc.tile_pool(name="xpool", bufs=2) as xp_pool,
                        tc.tile_pool(name="ps1", bufs=2, space="PSUM") as ps1,
                    ):
                        W1 = xp_pool.tile([64, 512], BF16, tag="w1")
                        nc.sync.dma_start(out=W1[:, :], in_=dins["W1"][:, :])
                        b1 = xp_pool.tile([128, 1], F32, tag="b1")
                        nc.sync.dma_start(out=b1[:, :], in_=dins["b1"][:, :])

                        chunks = [(0, 7), (7, 7), (14, 7), (21, 7), (28, 4),
                                  (32, 7), (39, 7), (46, 7), (53, 7), (60, 5)]
                        for half in range(2):
                            xt = xp_pool.tile([64, 68 * IMGP], BF16, tag="x")
                            src_off = 0 if half == 0 else 64 * IMGP
                            nc.sync.dma_start(
                                out=xt[:, :33 * IMGP],
                                in_=dins["xp"][:, src_off:src_off + 33 * IMGP])
                            nc.scalar.dma_start(
                                out=xt[:, 33 * IMGP:66 * IMGP],
                                in_=dins["xp"][:, src_off + 33 * IMGP:
                                               src_off + 66 * IMGP])
                            nc.gpsimd.memset(xt[:, 66 * IMGP:].bitcast(F32), 0.0)
                            row0 = 0 if half == 0 else 64
                            for c0, nr in chunks:
                                if (half == 0) != (c0 < 32):
                                    continue
                                pt = ps1.tile([128, 7 * 66], F32, tag="ps1")
                                for t, (ky, kx) in enumerate(
                                        [(0, 0), (0, 1), (1, 0), (1, 1)]):
                                    rhs = ap(xt, 0, 64,
                                             (2 * c0 + ky - row0) * IMGP + kx,
                                             [[2 * IMGP, nr], [2, 66]])
                                    nc.tensor.matmul(
                                        pt[:, :nr * 66],
                                        W1[:, t * 128:(t + 1) * 128],
                                        rhs, start=(t == 0), stop=(t == 3))
                                dst = ap(y_sb, 0, 128, (c0 + 1) * HP + 1,
                                         [[HP, nr], [1, H]])
                                nc.scalar.activation(dst,
                                                     ap(pt, 0, 128, 0,
                                                        [[66, nr], [1, H]]),
                                                     AF.Identity, bias=b1[:, :])

                    # ===== S2: rcv conv (q compact bf16 + v padded f32r) ====
                    with (
                        tc.tile_pool(name="qcpool", bufs=1) as qcp,
                        tc.tile_pool(name="ps2", bufs=2, space="PSUM") as ps2,
                    ):
                        q_sb = qcp.tile([128, NPIX], BF16, tag="qc")
                        for c0 in range(0, H, 7):
                            nr = min(7, H - c0)
                            pt = ps2.tile([128, 7 * 66], F32, tag="ps2")
                            for t in range(9):
                                ky, kx = t // 3, t % 3
                                rhs = ap(y_sb, 0, 128, (c0 + ky) * HP + kx,
                                         [[HP, nr], [1, 66]])
                                nc.tensor.matmul(
                                    pt[:, :nr * 66],
                                    Wrcv[:, t * 256:t * 256 + 128],
                                    rhs, start=(t == 0), stop=(t == 8))
                            nc.scalar.activation(q_sb[:, c0 * H:(c0 + nr) * H],
                                                 ap(pt, 0, 128, 0,
                                                    [[66, nr], [1, H]]),
                                                 AF.Identity, bias=brg1[:, :])
                            pt2 = ps2.tile([128, 7 * 66], F32, tag="ps2")
                            for t in range(9):
                                ky, kx = t // 3, t % 3
                                rhs = ap(y_sb, 0, 128, (c0 + ky) * HP + kx,
                                         [[HP, nr], [1, 66]])
                                nc.tensor.matmul(
                                    pt2[:, :nr * 66],
                                    Wrcv[:, t * 256 + 128:t * 256 + 256],
                                    rhs, start=(t == 0), stop=(t == 8))
                            dstv = ap(v_sb, 0, 128, (c0 + 1) * HP + 1,
                                      [[HP, nr], [1, H]])
                            nc.scalar.activation(dstv,
                                                 ap(pt2, 0, 128, 0,
                                                    [[66, nr], [1, H]]),
                                                 AF.Identity, bias=brg2[:, :])
                        # reshuffle q -> 32-aligned padded tensors (sbuf
                        # dma, spread across DGE queues to parallelize issue)
                        qeng = [nc.sync, nc.scalar]
                        for hi in range(4):
                            for blk, dstq in enumerate([qQ, qK, qC1, qC2]):
                                qeng[(hi * 4 + blk) % 2].dma_start(
                                    out=ap(dstq, 32 * hi, 8, 0, [[1, NPIX]]),
                                    in_=q_sb[blk * 32 + 8 * hi:
                                             blk * 32 + 8 * hi + 8, :])

                # ===== S3-S6: attention, software-pipelined per head =====
                # Per-head prologue (scores/exp/Z/iz/V-permute) is emitted as
                # generator steps interleaved into the PREVIOUS head's chunk
                # loop, so PE-heavy score work overlaps DVE/Pool-heavy chunks.
                with (
                    tc.tile_pool(name="hpool", bufs=2) as hp,
                    tc.tile_pool(name="mpool", bufs=5) as mp,
                    tc.tile_pool(name="up2pool", bufs=4) as up2,
                    tc.tile_pool(name="mhpool", bufs=6) as mhp,
                    tc.tile_pool(name="m2pool", bufs=2) as m2p,
                    tc.tile_pool(name="tpool", bufs=12) as tp,
                    tc.tile_pool(name="scps", bufs=1, space="PSUM") as scps,
                    tc.tile_pool(name="ups", bufs=2, space="PSUM") as ups,
                    tc.tile_pool(name="pttps", bufs=1, space="PSUM") as pttp,
                ):
                    _padded_heads = set()
                    _padded_m = set()

                    def alloc_head():
                        t = {}
                        for nm, shape, dt_ in [
                            ("er", [65, 66 * H], BF16),      # (w, i) 66-stride
                            ("er2", [128, 66 * H], BF16),
                            ("ertail", [65, 65], BF16),
                            ("ecs", [65, NECS], BF16),
                            ("ectail", [65, 65], BF16),
                            ("zf1", [65, 34 * H], BF16),     # Zr fold1 (w,34)
                            ("zravg", [65, 65], F32),
                            ("zrc", [128, 34], F32),
                            ("zcc", [128, 34], F32),
                            ("iz2", [128, 34], F32),
                            ("vpt", [65, ID16], BF16),
                        ]:
                            tl = hp.tile(shape, dt_, tag=nm)
                            t[nm] = tl
                        # zero pad columns read by the fold chains (col 65
                        # of each er 66-block; col 33 of zf1; col 17 of zf2)
                        if t["er"].offset not in _padded_heads:
                            _padded_heads.add(t["er"].offset)
                            nc.gpsimd.memset(
                                ap(t["er"], 0, 65, 65, [[66, H], [1, 1]]), 0.0)
                            nc.gpsimd.memset(
                                ap(t["zf1"], 0, 65, 33, [[34, H], [1, 1]]), 0.0)
                        return t

                    def prologue_steps(hi, t):
                        """Generator: emits one instruction group per next()."""
                        tpos = (32 * hi, 0)
                        er, ecs, ectail = t["er"], t["ecs"], t["ectail"]
                        # V-permute first (only needs v_sb)
                        for i0 in range(0, H, 32):
                            ni = min(32, H - i0)
                            ptv = scps.tile([128, 512], F32, tag="sc")
                            for k in range(ni):
                                i = i0 + k
                                src = ap(v_sb, 32 * hi, 16,
                                         (i + 1) * HP + 1, [[1, H]])
                                idn = ap(ident, 32 * hi, 16, 32 * hi,
                                         [[1, 16]])
                                nc.tensor.transpose(
                                    r32(ap(ptv, 0, 65, k * 16, [[1, 16]])),
                                    src, idn, tile_position=tpos)
                            nc.scalar.activation(
                                ap(t["vpt"], 0, 65, i0, [[1, ni], [H, 16]]),
                                ptv[:65, :ni * 16], AF.Copy)
                            yield
                        # r scores: per w -> psum [h, i]; exp -> er (bf16,
                        # 66-stride per w so the Zr fold chain needs no tail)
                        for w0 in range(0, H, 7):
                            nw = min(7, H - w0)
                            pt = scps_cur[0].tile([128, 512], F32, tag="sc")
                            for k in range(nw):
                                w = w0 + k
                                nc.tensor.matmul(
                                    pt[:65, k * 66:k * 66 + 66],
                                    ap(qK, 32 * hi, 8, w, [[H, H]]),
                                    ap(qQ, 32 * hi, 8, w, [[H, 66]]),
                                    start=True, stop=True,
                                    tile_position=tpos)
                            nc.scalar.activation(
                                ap(er, 0, 65, w0 * 66, [[66, nw], [1, H]]),
                                ap(pt, 0, 65, 0, [[66, nw], [1, H]]),
                                AF.Exp)
                            yield
                        # er2 (pixel-partitioned) + ertail
                        nc.scalar.dma_start(out=t["er2"][0:64, :],
                                            in_=er[0:64, :])
                        yield
                        nc.scalar.dma_start(out=t["er2"][64:128,
                                                         :66 * H - 66],
                                            in_=er[0:64, 66:])
                        yield
                        nc.scalar.dma_start(out=t["ertail"][:, :],
                                            in_=ap(er, 64, 1, 0,
                                                   [[66, H], [1, H]]))
                        yield
                        # Zr: fold (w,66)->(w,34) on Pool in 4 pieces, then
                        # DVE reduce-34 -> zravg[h,w]
                        zf1 = t["zf1"]
                        for wz0 in range(0, H, 17):
                            nwz = min(17, H - wz0)
                            pool_fold(
                                ap(zf1, 0, 65, wz0 * 34, [[34, nwz], [1, 33]]),
                                ap(er, 0, 65, wz0 * 66, [[66, nwz], [1, 33]]),
                                ap(er, 0, 65, wz0 * 66 + 33,
                                   [[66, nwz], [1, 33]]))
                            yield
                        nc.vector.tensor_reduce(
                            t["zravg"][:, :],
                            ap(zf1, 0, 65, 0, [[34, H], [1, 34]]),
                            AX.X, AluOpType.add)
                        yield
                        # c scores: per h -> psum [j, w]; exp -> ecs (f32r)
                        for h0 in range(0, H, 7):
                            nh = min(7, H - h0)
                            pt = scps_cur[0].tile([128, 512], F32, tag="sc")
                            for k in range(nh):
                                h = h0 + k
                                nc.tensor.matmul(
                                    pt[:65, k * 66:k * 66 + 66],
                                    ap(qC1, 32 * hi, 8, h * H, [[1, H]]),
                                    ap(qC2, 32 * hi, 8, h * H, [[1, 66]]),
                                    start=True, stop=True,
                                    tile_position=tpos)
                            nhs = min(nh, 64 - h0)
                            nc.scalar.activation(
                                ap(ecs, 0, 65, h0, [[1, nhs], [64, H]]),
                                ap(pt, 0, 65, 0, [[66, nhs], [1, H]]),
                                AF.Exp)
                            if h0 + nh == 65:
                                nc.scalar.activation(
                                    ectail[:, :],
                                    ap(pt, 0, 65, (nh - 1) * 66, [[1, H]]),
                                    AF.Exp)
                            yield
                        # Zc per chunk (chunk-partitioned ones-matmuls)
                        zct = scps_cur[0].tile([128, 512], F32, tag="sc")
                        for wb0 in range(0, 32, 16):
                            for wb in range(wb0, wb0 + 16):
                                nc.tensor.matmul(
                                    ap(zct, 0, 128, 2 * wb, [[1, 2]]),
                                    ap(ecs, 0, 65, wb * 128, [[1, 128]]),
                                    ones65[:, :], start=True, stop=True)
                            yield
                        nc.tensor.matmul(
                            ap(zct, 0, 64, 64, [[1, 2]]),
                            ap(ecs, 0, 65, 64 * 64, [[1, 64]]),
                            ones65[:, :], start=True, stop=True)
                        nc.tensor.matmul(
                            ap(zct, 0, 65, 66, [[1, 2]]),
                            ap(ectail, 0, 65, 0, [[1, H]]),
                            ones65[:, :], start=True, stop=True)
                        zcc = t["zcc"]
                        nc.scalar.activation(zcc[0:64, :],
                                             ap(zct, 0, 64, 0, [[2, 34]]),
                                             AF.Copy)
                        nc.scalar.activation(zcc[64:128, 0:32],
                                             ap(zct, 64, 64, 0, [[2, 32]]),
                                             AF.Copy)
                        nc.scalar.activation(zcc[64:65, 33:34],
                                             ap(zct, 64, 1, 66, [[1, 1]]),
                                             AF.Copy)
                        yield
                        # zr chunk columns + iz scalars
                        zravg, zrc = t["zravg"], t["zrc"]
                        nc.vector.tensor_copy(
                            ap(zrc, 0, 64, 0, [[1, 32]]),
                            ap(zravg, 0, 64, 0, [[2, 32]]))
                        nc.vector.tensor_copy(
                            ap(zrc, 64, 64, 0, [[1, 32]]),
                            ap(zravg, 0, 64, 1, [[2, 32]]))
                        nc.vector.tensor_copy(zrc[0:64, 32:33],
                                              zravg[0:64, 64:65])
                        nc.scalar.dma_start(out=zrc[0:65, 33:34],
                                            in_=ap(zravg, 64, 1, 0, [[1, H]]))
                        iz2 = t["iz2"]
                        nc.vector.tensor_tensor(out=iz2[:, :], in0=zrc[:, :],
                                                in1=zcc[:, :],
                                                op=AluOpType.mult)
                        nc.vector.reciprocal(iz2[:, :], iz2[:, :])
                        yield

                    def pe_p_steps():
                        """S7 pe-conv into A_sb rows 64:128 (needs v_sb)."""
                        for w0 in range(0, H, 7):
                            nw = min(7, H - w0)
                            pt = scps_cur[0].tile([128, 512], F32, tag="sc")
                            for tt in range(9):
                                ky, kx = tt // 3, tt % 3
                                rhs = ap(v_sb, 0, 128, ky * HP + kx + w0,
                                         [[1, nw], [HP, 66]])
                                nc.tensor.matmul(
                                    pt[:64, :nw * 66],
                                    Wpe[:, tt * 64:tt * 64 + 64],
                                    rhs, start=(tt == 0), stop=(tt == 8))
                            nc.scalar.activation(
                                ap(A_sb, 64, 64, w0 * H, [[1, nw * H]]),
                                ap(pt, 0, 64, 0, [[66, nw], [1, H]]),
                                AF.Identity, bias=bpe[:, :])
                            yield

                    state = {"ptt": None, "off": 0}
                    pend = []

                    def pool_fold(out, in0, in1):
                        nc.gpsimd.tensor_tensor(out=out, in0=in0, in1=in1,
                                                op=AluOpType.add)

                    def do_chunk(t, idx, lhsT_ap, er_ap, izcol, M, dst):
                        ut = ups.tile([128, ID16], F32, tag="ut")
                        for n0 in (0, 512, 1024):
                            nn = min(512, ID16 - n0)
                            nc.tensor.matmul(ut[:M, n0:n0 + nn],
                                             lhsT_ap,
                                             t["vpt"][:, n0:n0 + nn],
                                             start=True, stop=True)
                        # m is (d, i66)-strided bf16; col 65 of each d-block
                        # is pre-zeroed so the fold chain needs no tail ops
                        # E-chunks (DVE stt from PSUM) 1 in 4; fold2 on DVE
                        ph = 2 if idx % 4 == 0 else 0
                        m = mp.tile([128, 1066], BF16, tag="m")
                        if m.offset not in _padded_m:
                            _padded_m.add(m.offset)
                            nc.gpsimd.memset(
                                ap(m, 0, 128, 65, [[66, 16], [1, 1]]), 0.0)
                        if ph == 2:
                            # DVE: (ut * iz) * er straight out of PSUM (1x)
                            nc.vector.scalar_tensor_tensor(
                                out=ap(m, 0, M, 0, [[66, 16], [1, 65]]),
                                in0=ap(ut, 0, M, 0, [[65, 16], [1, 65]]),
                                scalar=t["iz2"][:M, izcol:izcol + 1],
                                in1=er_ap,
                                op0=AluOpType.mult, op1=AluOpType.mult)
                        else:
                            # Act drains PSUM to bf16 applying iz via scale;
                            # DVE multiplies by raw er at 2x
                            utb = up2.tile([128, ID16], BF16, tag="utb")
                            nc.scalar.activation(
                                ap(utb, 0, M, 0, [[65, 16], [1, 65]]),
                                ap(ut, 0, M, 0, [[65, 16], [1, 65]]),
                                AF.Identity,
                                scale=t["iz2"][:M, izcol:izcol + 1])
                            nc.vector.tensor_tensor(
                                out=ap(m, 0, M, 0, [[66, 16], [1, 65]]),
                                in0=ap(utb, 0, M, 0, [[65, 16], [1, 65]]),
                                in1=er_ap, op=AluOpType.mult)
                        # Pool fold 66 -> 34 via stt-add (col 33 pre-zeroed)
                        mh = mhp.tile([128, 544], BF16, tag="mh")
                        if mh.offset not in _padded_m:
                            _padded_m.add(mh.offset)
                            nc.gpsimd.memset(
                                ap(mh, 0, 128, 33, [[34, 16], [1, 1]]), 0.0)
                        pool_fold(ap(mh, 0, M, 0, [[34, 16], [1, 33]]),
                                  ap(m, 0, M, 0, [[66, 16], [1, 33]]),
                                  ap(m, 0, M, 33, [[66, 16], [1, 33]]))
                        # fold2+reduce deferred one chunk so DVE never
                        # stalls waiting on this chunk's Pool fold1
                        pend2.append((mh, M, dst))
                        if len(pend2) > 1:
                            do_tail(*pend2.pop(0))

                    def do_tail(mh, M, dst):
                        m2 = m2p.tile([128, 272], BF16, tag="m2")
                        nc.vector.tensor_tensor(
                            out=ap(m2, 0, M, 0, [[17, 16], [1, 17]]),
                            in0=ap(mh, 0, M, 0, [[34, 16], [1, 17]]),
                            in1=ap(mh, 0, M, 17, [[34, 16], [1, 17]]),
                            op=AluOpType.add)
                        at = tp.tile([128, 16], F32, tag="at")
                        nc.vector.tensor_reduce(
                            at[:M, :], ap(m2, 0, M, 0, [[17, 16], [1, 17]]),
                            AX.X, AluOpType.add)
                        pend.append((at, M, dst))

                    def emit_transpose(at, M):
                        if state["ptt"] is None:
                            pttt = pttp.tile([128, 512], F32, tag="ptt")
                            state["ptt"] = pttt
                            state["off"] = 0
                        nc.tensor.transpose(
                            ap(state["ptt"], 0, 16, state["off"], [[1, M]]),
                            at[:M, :], ident[:M, :M].bitcast(F32))
                        state["off"] += M

                    def drain_pend(keep):
                        while len(pend) > keep:
                            at, M, dst = pend.pop(0)
                            emit_transpose(at, M)
                            if dst is not None:
                                nc.scalar.activation(
                                    dst,
                                    ap(state["ptt"], 0, 16, 0,
                                       [[1, state["off"]]]),
                                    AF.Copy)
                                state["ptt"] = None

                    tiles = alloc_head()
                    for _ in prologue_steps(0, tiles):
                        pass
                    for hi in range(4):
                        A0 = 16 * hi
                        t = tiles
                        if hi < 3:
                            tiles = alloc_head()
                            nxt = prologue_steps(hi + 1, tiles)
                        else:
                            nxt = pe_p_steps()
                        er, ertail = t["er"], t["ertail"]
                        for wb in range(32):
                            dst = (ap(A_sb, A0, 16, 2 * (wb - 3) * H,
                                      [[H, 8], [1, 64]])
                                   if wb % 4 == 3 else None)
                            do_chunk(t, wb,
                                     ap(t["ecs"], 0, 65, wb * 128, [[1, 128]]),
                                     ap(t["er2"], 0, 128, 2 * wb * 66,
                                        [[0, 16], [1, H]]),
                                     wb, 128, dst)
                            drain_pend(8)
                            next(nxt, None)
                        do_chunk(t, 32,
                                 ap(t["ecs"], 0, 65, 64 * 64, [[1, 64]]),
                                 ap(t["er2"], 0, 64, 64 * 66,
                                    [[0, 16], [1, H]]),
                                 32, 64, ap(A_sb, A0, 16, 64 * H, [[1, 64]]))
                        drain_pend(8)
                        next(nxt, None)
                        do_chunk(t, 33,
                                 ap(t["ectail"], 0, 65, 0, [[1, H]]),
                                 ap(ertail, 0, 65, 0, [[0, 16], [1, H]]),
                                 33, 65, ap(A_sb, A0, 16, 64, [[H, H]]))
                        drain_pend(8)
                        for _ in nxt:
                            pass
                    drain_pend(0)

            # ===== S8: dconv(A) + dconv(P) -> dc3 rows 0:32 =====
            with (
                tc.tile_pool(name="dcpool", bufs=1) as dcp,
                tc.tile_pool(name="ps8", bufs=2, space="PSUM") as ps8,
            ):
                dc3 = dcp.tile([96, IMGP * IMGP], BF16, tag="dc3")
                # zero borders: block b holds dcpad rows shifted by b, so
                # block0 rows {0,129}, block1 rows {128,129-ish}, block2
                # rows {127,128}; plus the 1-px column strips everywhere.
                nc.gpsimd.dma_start(out=ap(dc3, 0, 32, 0, [[1, IMGP]]),
                                    in_=zbf[:32, :IMGP])
                nc.sync.dma_start(
                    out=ap(dc3, 0, 32, 129 * IMGP, [[1, IMGP]]),
                    in_=zbf[:32, :IMGP])
                nc.sync.dma_start(
                    out=ap(dc3, 32, 32, 128 * IMGP, [[1, 2 * IMGP]]),
                    in_=zbf[:32, :2 * IMGP])
                nc.sync.dma_start(
                    out=ap(dc3, 64, 32, 127 * IMGP, [[1, 3 * IMGP]]),
                    in_=zbf[:32, :3 * IMGP])
                for blk in range(3):
                    nc.sync.dma_start(
                        out=ap(dc3, 32 * blk, 32, IMGP, [[IMGP, 128], [1, 1]]),
                        in_=zbf[:32, :128])
                    nc.sync.dma_start(
                        out=ap(dc3, 32 * blk, 32, IMGP + 129,
                               [[IMGP, 128], [1, 1]]),
                        in_=zbf[:32, :128])
                # a0-outer so dc rows complete in ascending order; the
                # row-shifted copies for the 96-deep final conv are issued
                # piecewise so S9 can pipeline behind S8.
                shift_done = 0

                def dc3_shift_upto(row):
                    nonlocal shift_done
                    lo = shift_done
                    if row <= lo:
                        return
                    nc.sync.dma_start(
                        out=dc3[32:64, lo * IMGP:row * IMGP],
                        in_=dc3[0:32, (lo + 1) * IMGP:(row + 1) * IMGP])
                    nc.gpsimd.dma_start(
                        out=dc3[64:96, lo * IMGP:row * IMGP],
                        in_=dc3[0:32, (lo + 2) * IMGP:(row + 2) * IMGP])
                    shift_done = row

                for a0 in range(0, 64, 8):
                    for pr in range(2):
                        for ps in range(2):
                            pt = ps8.tile([32, 512], F32, tag="dcps")
                            w0 = (pr * 2 + ps) * 32
                            nc.tensor.matmul(
                                pt[:, :], Wdc[:, w0:w0 + 32],
                                ap(A_sb, 0, 128, ps * H + pr + a0,
                                   [[1, 8], [H, 64]]),
                                start=True, stop=True)
                            dst = ap(dc3, 0, 32,
                                     (2 * a0 + pr + 1) * IMGP + ps + 1,
                                     [[2 * IMGP, 8], [2, 64]])
                            if (pr * 2 + ps) % 2 == 0:
                                nc.scalar.activation(dst, pt[:, :],
                                                     AF.Identity,
                                                     bias=bdc[:, :])
                            else:
                                nc.vector.scalar_tensor_tensor(
                                    out=dst, in0=pt[:, :], scalar=1.0,
                                    in1=ap(bdc, 0, 32, 0, [[0, 8], [0, 64]]),
                                    op0=AluOpType.mult, op1=AluOpType.add)
                    if a0 in (24, 40, 56):
                        # rows complete up to 2*a0+16 after this block
                        dc3_shift_upto(2 * a0 + 14)
                nc.sync.dma_start(
                    out=dc3[32:64, shift_done * IMGP:IMGP * IMGP - IMGP],
                    in_=dc3[0:32, (shift_done + 1) * IMGP:])
                nc.gpsimd.dma_start(
                    out=dc3[64:96, shift_done * IMGP:IMGP * IMGP - 2 * IMGP],
                    in_=dc3[0:32, (shift_done + 2) * IMGP:IMGP * IMGP])

                # ===== S9: final conv partial, 96-deep =====
                with (
                    tc.tile_pool(name="opool", bufs=2) as op_,
                    tc.tile_pool(name="ps9", bufs=2, space="PSUM") as ps9,
                ):
                    ost = None
                    for r0 in range(0, IMG, 4):
                        pt = ps9.tile([64, 512], F32, tag="o")
                        for kx in range(3):
                            rhs = ap(dc3, 0, 96, r0 * IMGP + kx,
                                     [[IMGP, 4], [1, IMG]])
                            nc.tensor.matmul(pt[:, :],
                                             Wout3[:, kx * 64:kx * 64 + 64],
                                             rhs, start=(kx == 0),
                                             stop=(kx == 2))
                        if r0 % 16 == 0:
                            ost = op_.tile([64, 2048], BF16, tag="ost")
                        sl = (r0 % 16) // 4 * 512
                        if (r0 // 4) % 2 == 0:
                            nc.scalar.activation(ost[:, sl:sl + 512], pt[:, :],
                                                 AF.Identity, bias=bfin[:, :])
                        else:
                            nc.vector.scalar_tensor_tensor(
                                out=ost[:, sl:sl + 512], in0=pt[:, :],
                                scalar=1.0,
                                in1=ap(bfin, 0, 64, 0, [[0, 512]]),
                                op0=AluOpType.mult, op1=AluOpType.add)
                        if r0 % 16 == 12:
                            oeng = nc.sync if r0 % 32 == 12 else nc.gpsimd
                            oeng.dma_start(out=out_d[:, r0 - 12:r0 + 4, :],
                                           in_=ost[:, :])

    nc.compile()
    return nc


_NC_CACHE = None


def kernel(**inputs):
    global _NC_CACHE
    if _NC_CACHE is None:
        _NC_CACHE = build_nc()
    nc = _NC_CACHE
    in_maps = [prep_core_inputs(inputs, c // 2, c % 2) for c in range(8)]
    res = run_bass_kernel_spmd(nc, in_maps, list(range(8)))
    out = np.zeros((4, 64, IMG, IMG), np.float32)
    for b in range(4):
        out[b] = (res.results[2 * b]["out"].astype(np.float32) +
                  res.results[2 * b + 1]["out"].astype(np.float32))
    return out



# revision 42
# speedup vs baseline: 1.1852x; 1.0008x over previous
"""Trainium2 Bass kernel for nn_MatrixAttention (sparse_attention).

Sharding: 8 cores = (batch b in 0..3) x (head-group g in 0..1, 4 heads each).
Each core: in_proj -> rcv conv (its 192 ch) -> row/col attention (4 heads)
-> pe conv -> grouped deconv (its 32 dc ch) -> partial final 3x3 conv over
all 64 output channels from its 32 dc channels. Host gather sums the pair
partials (input-dim-sharded conv => reduce-gather) and stacks batches.

Perf structure (TimelineSim-tuned):
- q/ecs/vpt/scores all bf16 (f32r matmuls with <256-col outputs pay a 4x
  cycle penalty; bf16 is 1 cycle/row and halves SBUF).
- S2 split: G1 (q) chunks first, then one merged q-reshuffle (sync+gpsimd
  DGE queues; the ~630ns/DMA descriptor-gen serializes on HWDGE, so it
  must never sit on the Act/SP queues mid-pipeline), then G2 (v) chunks
  with the head-0 prologue generator interleaved (scores/exp/Zc/Zr/iz
  hide behind G2's PE work; V-permute last since it needs v).
- Combine chunks (128 px): PE matmul (ec^T V, 65-deep, 1040 cols) ->
  D-chunks: Act drain (iz scale) to bf16 + DVE tensor_tensor x er (2x) |
  E-chunks (1 in 4): DVE stt from PSUM (1x) -> Pool fold 65->33 ->
  DVE reduce-33 -> PE transpose -> Act flush into A.
  The fold2/reduce of chunk N runs at the TOP of chunk N+2 (pend2) and
  transposes/flushes are deferred 4 chunks (pend): in-order engine queues
  otherwise serialize the whole chain per chunk.
- Zr via Pool fold (w,66-stride er, pad col zeroed per head) + DVE
  reduce-33; Zc via ones-matmuls; iz=1/(Zr*Zc) as drain scale/stt scalar.
- S8/S9 drains alternate Act / DVE-stt(+bias broadcast); output stores
  batched 4 row-groups per DMA (HWDGE descriptor-gen is the tail limit).

Self-contained: hardcodes all shapes; no sibling imports.
"""
import sys
import numpy as np

sys.path.insert(0, "/opt/trn_rl_repo")

import ml_dtypes                        # noqa: E402
import concourse.bass as bass           # noqa: E402
import concourse.bacc as bacc           # noqa: E402
import concourse.mybir as mybir         # noqa: E402
from concourse.tile import TileContext  # noqa: E402
from concourse.bass_utils import run_bass_kernel_spmd  # noqa: E402
from concourse.alu_op_type import AluOpType  # noqa: E402

F32 = mybir.dt.float32
F32R = mybir.dt.float32r
BF16 = mybir.dt.bfloat16
AF = mybir.ActivationFunctionType
AX = mybir.AxisListType
BF16NP = ml_dtypes.bfloat16

NH, KD, HD = 8, 8, 16
SCALE = KD ** -0.5
H = 65            # spatial after in_proj
HP = 67           # padded
NPIX = H * H      # 4225
PADPIX = HP * HP  # 4489
IMG = 128
IMGP = 130
ID16 = 1040       # (i,d) = 65*16
NECS = 65 * 64    # 4160: w-major (h<64) ec storage


def r32(x):
    return x.bitcast(F32R)


def ap(tile, part0, nparts, free_off, free_dims):
    """AP over a tile: partitions [part0, part0+nparts), free offset + dims
    (list of [step, count], outer->inner)."""
    pitch = tile.ap[0][0]
    return bass.AP(tile.tensor, tile.offset + part0 * pitch + free_off,
                   [[pitch, nparts]] + [list(d) for d in free_dims])


# ----------------------------------------------------------------------------
# Host-side weight prep
# ----------------------------------------------------------------------------
def prep_core_inputs(inputs, b, g):
    inp = {k: np.ascontiguousarray(np.asarray(v), dtype=np.float32)
           for k, v in inputs.items()}
    heads = list(range(4 * g, 4 * g + 4))

    xp = np.zeros((64, IMGP, IMGP), np.float32)
    xp[:, 1:129, 1:129] = inp["x"][b]
    xp = xp.reshape(64, IMGP * IMGP)

    W1 = np.zeros((2, 2, 64, 128), np.float32)
    for co in range(128):
        W1[:, :, co // 2, co] = inp["w_in"][co, 0] * inp["s_in"][co]
    W1 = W1.reshape(4, 64, 128).transpose(1, 0, 2).reshape(64, 512)
    b1 = inp["b_in"].reshape(128, 1)

    # rcv conv weights. G1 (compact q): cols = [rq 4hx8 | rk | cq | ck].
    # G2 (v, padded): col 32*hi + dd  holds v-channel dd of head hi.
    w_rcv = inp["w_rcv"] * inp["s_rcv"][:, None, None, None]
    qrows = []
    for blk in range(4):           # rq, rk, cq, ck
        for h in heads:
            qrows.extend(range(h * 48 + blk * 8, h * 48 + blk * 8 + 8))
    Wq = w_rcv[qrows]              # [128, 128, 3, 3]
    bq = inp["b_rcv"][qrows].copy()
    scale_mask = np.ones(128, np.float32)
    scale_mask[0:32] = SCALE       # rq
    scale_mask[64:96] = SCALE      # cq
    Wq = Wq * scale_mask[:, None, None, None]
    bq = bq * scale_mask
    Wv = np.zeros((128, 128, 3, 3), np.float32)   # padded v rows
    bv = np.zeros((128, 1), np.float32)
    for hi, h in enumerate(heads):
        for dd in range(16):
            Wv[32 * hi + dd] = w_rcv[h * 48 + 32 + dd]
            bv[32 * hi + dd, 0] = inp["b_rcv"][h * 48 + 32 + dd]
    # lhsT [ci=128, 9 taps, 256 cols (G1 128 | G2 128)]
    Wrcv = np.concatenate(
        [Wq.transpose(1, 2, 3, 0).reshape(128, 9, 128),
         Wv.transpose(1, 2, 3, 0).reshape(128, 9, 128)], axis=2
    ).reshape(128, 9 * 256)
    brcv_g1 = bq.reshape(128, 1)
    brcv_g2 = bv

    # pe conv: input/output both padded to 128 (head hi at rows/cols 32*hi)
    w_pe = inp["w_pe"] * inp["s_pe"][:, None, None, None]
    Wpe = np.zeros((128, 3, 3, 64), np.float32)
    bpe = np.zeros((64, 1), np.float32)
    for hi, h_abs in enumerate(heads):
        for col in range(16):
            co = h_abs * 16 + col
            col_l = 16 * hi + col
            for k in range(2):
                ci_row = 32 * hi + 2 * (col // 2) + k
                Wpe[ci_row, :, :, col_l] = w_pe[co, k]
            bpe[col_l, 0] = inp["b_pe"][co]
    Wpe = Wpe.reshape(128, 9 * 64)

    w_dc = inp["w_dc"]
    g0 = heads[0] * 8
    # rows: A-compact channels 0:64, P-compact channels 64:128 (dconv of
    # A+P done as one 128-deep matmul over the merged AP tile)
    Wdc = np.zeros((128, 2, 2, 32), np.float32)
    bdc = np.zeros((32, 1), np.float32)
    for cl in range(32):
        co = g0 + cl
        hi, c = cl // 8, cl % 8
        for k in range(2):
            Wdc[16 * hi + 2 * c + k, :, :, cl] = w_dc[co, k]
            Wdc[64 + 16 * hi + 2 * c + k, :, :, cl] = w_dc[co, k]
        bdc[cl, 0] = inp["b_dc"][co]
    Wdc = Wdc.reshape(128, 4 * 32)

    # final conv, 96-deep (ky folded into contraction): rows (ky, ci32),
    # cols (kx, co64)
    w_out = inp["w_out"] * inp["s_out"][:, None, None, None]   # [64,64,3,3]
    Wout3 = np.zeros((96, 3, 64), np.float32)
    for ky in range(3):
        for ci in range(32):
            for kx in range(3):
                Wout3[ky * 32 + ci, kx, :] = w_out[:, 32 * g + ci, ky, kx]
    Wout3 = Wout3.reshape(96, 192)
    bfin = (inp["b_out"] if g == 0 else np.zeros(64, np.float32)).reshape(64, 1)

    return {
        "xp": xp.astype(BF16NP), "W1": np.ascontiguousarray(W1).astype(BF16NP), "b1": b1,
        "Wrcv": np.ascontiguousarray(Wrcv).astype(BF16NP),
        "brcv_g1": brcv_g1, "brcv_g2": brcv_g2,
        "Wpe": np.ascontiguousarray(Wpe), "bpe": bpe,
        "Wdc": np.ascontiguousarray(Wdc).astype(BF16NP), "bdc": bdc,
        "Wout3": np.ascontiguousarray(Wout3).astype(BF16NP), "bfin": bfin,
        "ident": np.eye(128, dtype=np.float32),
        "ones": np.ones((65, 2), np.float32).astype(BF16NP),
        "zeros": np.zeros((128, PADPIX), np.float32),
    }


# ----------------------------------------------------------------------------
# Device program
# ----------------------------------------------------------------------------
def build_nc():
    nc = bacc.Bacc(None, target_bir_lowering=False)

    dins = {}
    for name, shape, dt_ in [
        ("xp", [64, IMGP * IMGP], BF16), ("W1", [64, 512], BF16),
        ("b1", [128, 1], F32),
        ("Wrcv", [128, 2304], BF16), ("brcv_g1", [128, 1], F32),
        ("brcv_g2", [128, 1], F32),
        ("Wpe", [128, 576], F32R), ("bpe", [64, 1], F32),
        ("Wdc", [128, 128], BF16), ("bdc", [32, 1], F32),
        ("Wout3", [96, 192], BF16), ("bfin", [64, 1], F32),
        ("ident", [128, 128], F32R),
        ("ones", [65, 2], BF16),
        ("zeros", [128, PADPIX], F32R),
    ]:
        dins[name] = nc.dram_tensor(name, shape, dt_, kind="ExternalInput")
    out_d = nc.dram_tensor("out", [64, IMG, IMG], BF16, kind="ExternalOutput")
    zbf = dins["zeros"].bitcast(BF16)   # [128, 2*PADPIX] of bf16 zeros

    with TileContext(nc) as tc:
        with (
            tc.tile_pool(name="wpool", bufs=1) as wp,
            tc.tile_pool(name="vpool", bufs=1) as vp_,
            tc.tile_pool(name="apool", bufs=1) as ap_,
        ):
            def load(name, shape, dt_=F32):
                t = wp.tile(shape, dt_, tag=name)
                # weights go on the Pool SWDGE queue so the x/W1 loads on
                # the SP/Act HWDGE queues start immediately
                eng = nc.gpsimd if shape[0] * shape[1] > 4096 else nc.sync
                eng.dma_start(out=t[:, :], in_=dins[name][:, :])
                return t

            Wrcv = load("Wrcv", [128, 2304], BF16)
            brg1 = load("brcv_g1", [128, 1])
            brg2 = load("brcv_g2", [128, 1])
            Wpe = load("Wpe", [128, 576], F32R)
            bpe = load("bpe", [64, 1])
            Wdc = load("Wdc", [128, 128], BF16)
            bdc = load("bdc", [32, 1])
            Wout3 = load("Wout3", [96, 192], BF16)
            bfin = load("bfin", [64, 1])
            ident = load("ident", [128, 128], F32R)
            ones65 = load("ones", [65, 2], BF16)

            v_sb = vp_.tile([128, PADPIX + 2 * HP], F32R, tag="v")  # (h,w) pad
            nc.vector.memset(v_sb[:, :].bitcast(F32), 0.0)
            # merged tile: rows 0:64 = attention A (16 per head, compact),
            # rows 64:128 = pe-conv P (compact); no pad rows
            A_sb = ap_.tile([128, NPIX], BF16, tag="A")      # (w,h)-major

            with tc.tile_pool(name="qxpool", bufs=1) as qx:
                qQ = qx.tile([128, NPIX + H], BF16, tag="qQ")
                qK = qx.tile([128, NPIX + H], BF16, tag="qK")
                qC1 = qx.tile([128, NPIX + H], BF16, tag="qC1")
                qC2 = qx.tile([128, NPIX + H], BF16, tag="qC2")
                for _t in (qQ, qK, qC1, qC2):
                    nc.gpsimd.memset(_t[:, NPIX:], 0.0)

                with tc.tile_pool(name="ypool", bufs=1) as yp:
                    y_sb = yp.tile([128, PADPIX + 2 * HP + 1], BF16, tag="y")
                    nc.gpsimd.memset(y_sb[:, :].bitcast(F32), 0.0)

                    # ===== S1: in_proj (x loaded in two halves) =====
                    with (
                        tc.tile_pool(name="xpool", bufs=2) as xp_pool,
                        tc.tile_pool(name="ps1", bufs=2, space="PSUM") as ps1,
                    ):
                        W1 = xp_pool.tile([64, 512], BF16, tag="w1")
                        nc.sync.dma_start(out=W1[:, :], in_=dins["W1"][:, :])
                        b1 = xp_pool.tile([128, 1], F32, tag="b1")
                        nc.sync.dma_start(out=b1[:, :], in_=dins["b1"][:, :])

                        chunks = [(0, 7), (7, 7), (14, 7), (21, 7), (28, 4),
                                  (32, 7), (39, 7), (46, 7), (53, 7), (60, 5)]
                        for half in range(2):
                            xt = xp_pool.tile([64, 68 * IMGP], BF16, tag="x")
                            src_off = 0 if half == 0 else 64 * IMGP
                            nc.sync.dma_start(
                                out=xt[:, :33 * IMGP],
                                in_=dins["xp"][:, src_off:src_off + 33 * IMGP])
                            nc.sync.dma_start(
                                out=xt[:, 33 * IMGP:66 * IMGP],
                                in_=dins["xp"][:, src_off + 33 * IMGP:
                                               src_off + 66 * IMGP])
                            nc.gpsimd.memset(xt[:, 66 * IMGP:].bitcast(F32), 0.0)
                            row0 = 0 if half == 0 else 64
                            for c0, nr in chunks:
                                if (half == 0) != (c0 < 32):
                                    continue
                                pt = ps1.tile([128, 7 * 66], F32, tag="ps1")
                                for t, (ky, kx) in enumerate(
                                        [(0, 0), (0, 1), (1, 0), (1, 1)]):
                                    rhs = ap(xt, 0, 64,
                                             (2 * c0 + ky - row0) * IMGP + kx,
                                             [[2 * IMGP, nr], [2, 66]])
                                    nc.tensor.matmul(
                                        pt[:, :nr * 66],
                                        W1[:, t * 128:(t + 1) * 128],
                                        rhs, start=(t == 0), stop=(t == 3))
                                dst = ap(y_sb, 0, 128, (c0 + 1) * HP + 1,
                                         [[HP, nr], [1, H]])
                                nc.scalar.activation(dst,
                                                     ap(pt, 0, 128, 0,
                                                        [[66, nr], [1, H]]),
                                                     AF.Identity, bias=b1[:, :])

                    # ===== S2: rcv conv (q compact bf16 + v padded f32r) ====
                    with (
                        tc.tile_pool(name="qcpool", bufs=1) as qcp,
                        tc.tile_pool(name="ps2", bufs=2, space="PSUM") as ps2,
                    ):
                        q_sb = qcp.tile([128, NPIX], BF16, tag="qc")
                        for c0 in range(0, H, 7):
                            nr = min(7, H - c0)
                            pt = ps2.tile([128, 7 * 66], F32, tag="ps2")
                            for t in range(9):
                                ky, kx = t // 3, t % 3
                                rhs = ap(y_sb, 0, 128, (c0 + ky) * HP + kx,
                                         [[HP, nr], [1, 66]])
                                nc.tensor.matmul(
                                    pt[:, :nr * 66],
                                    Wrcv[:, t * 256:t * 256 + 128],
                                    rhs, start=(t == 0), stop=(t == 8))
                            nc.scalar.activation(q_sb[:, c0 * H:(c0 + nr) * H],
                                                 ap(pt, 0, 128, 0,
                                                    [[66, nr], [1, H]]),
                                                 AF.Identity, bias=brg1[:, :])
                            pt2 = ps2.tile([128, 7 * 66], F32, tag="ps2")
                            for t in range(9):
                                ky, kx = t // 3, t % 3
                                rhs = ap(y_sb, 0, 128, (c0 + ky) * HP + kx,
                                         [[HP, nr], [1, 66]])
                                nc.tensor.matmul(
                                    pt2[:, :nr * 66],
                                    Wrcv[:, t * 256 + 128:t * 256 + 256],
                                    rhs, start=(t == 0), stop=(t == 8))
                            dstv = ap(v_sb, 0, 128, (c0 + 1) * HP + 1,
                                      [[HP, nr], [1, H]])
                            nc.scalar.activation(dstv,
                                                 ap(pt2, 0, 128, 0,
                                                    [[66, nr], [1, H]]),
                                                 AF.Identity, bias=brg2[:, :])
                        # reshuffle q -> 32-aligned padded tensors (sbuf
                        # dma, spread across DGE queues to parallelize issue)
                        qeng = [nc.sync, nc.scalar]
                        for hi in range(4):
                            for blk, dstq in enumerate([qQ, qK, qC1, qC2]):
                                qeng[(hi * 4 + blk) % 2].dma_start(
                                    out=ap(dstq, 32 * hi, 8, 0, [[1, NPIX]]),
                                    in_=q_sb[blk * 32 + 8 * hi:
                                             blk * 32 + 8 * hi + 8, :])

                # ===== S3-S6: attention, software-pipelined per head =====
                # Per-head prologue (scores/exp/Z/iz/V-permute) is emitted as
                # generator steps interleaved into the PREVIOUS head's chunk
                # loop, so PE-heavy score work overlaps DVE/Pool-heavy chunks.
                with (
                    tc.tile_pool(name="hpool", bufs=2) as hp,
                    tc.tile_pool(name="mpool", bufs=5) as mp,
                    tc.tile_pool(name="up2pool", bufs=4) as up2,
                    tc.tile_pool(name="mhpool", bufs=6) as mhp,
                    tc.tile_pool(name="m2pool", bufs=2) as m2p,
                    tc.tile_pool(name="tpool", bufs=12) as tp,
                    tc.tile_pool(name="scps", bufs=1, space="PSUM") as scps,
                    tc.tile_pool(name="ups", bufs=2, space="PSUM") as ups,
                    tc.tile_pool(name="pttps", bufs=1, space="PSUM") as pttp,
                ):
                    _padded_heads = set()
                    _padded_m = set()

                    def alloc_head():
                        t = {}
                        for nm, shape, dt_ in [
                            ("er", [65, 66 * H], BF16),      # (w, i) 66-stride
                            ("er2", [128, 66 * H], BF16),
                            ("ertail", [65, 65], BF16),
                            ("ecs", [65, NECS], BF16),
                            ("ectail", [65, 65], BF16),
                            ("zf1", [65, 34 * H], BF16),     # Zr fold1 (w,34)
                            ("zravg", [65, 65], F32),
                            ("zrc", [128, 34], F32),
                            ("zcc", [128, 34], F32),
                            ("iz2", [128, 34], F32),
                            ("vpt", [65, ID16], BF16),
                        ]:
                            tl = hp.tile(shape, dt_, tag=nm)
                            t[nm] = tl
                        # zero pad columns read by the fold chains (col 65
                        # of each er 66-block; col 33 of zf1; col 17 of zf2)
                        if t["er"].offset not in _padded_heads:
                            _padded_heads.add(t["er"].offset)
                            nc.gpsimd.memset(
                                ap(t["er"], 0, 65, 65, [[66, H], [1, 1]]), 0.0)
                            nc.gpsimd.memset(
                                ap(t["zf1"], 0, 65, 33, [[34, H], [1, 1]]), 0.0)
                        return t

                    def prologue_steps(hi, t):
                        """Generator: emits one instruction group per next()."""
                        tpos = (32 * hi, 0)
                        er, ecs, ectail = t["er"], t["ecs"], t["ectail"]
                        # V-permute first (only needs v_sb)
                        for i0 in range(0, H, 32):
                            ni = min(32, H - i0)
                            ptv = scps.tile([128, 512], F32, tag="sc")
                            for k in range(ni):
                                i = i0 + k
                                src = ap(v_sb, 32 * hi, 16,
                                         (i + 1) * HP + 1, [[1, H]])
                                idn = ap(ident, 32 * hi, 16, 32 * hi,
                                         [[1, 16]])
                                nc.tensor.transpose(
                                    r32(ap(ptv, 0, 65, k * 16, [[1, 16]])),
                                    src, idn, tile_position=tpos)
                            nc.scalar.activation(
                                ap(t["vpt"], 0, 65, i0, [[1, ni], [H, 16]]),
                                ptv[:65, :ni * 16], AF.Copy)
                            yield
                        # r scores: per w -> psum [h, i]; exp -> er (bf16,
                        # 66-stride per w so the Zr fold chain needs no tail)
                        for w0 in range(0, H, 7):
                            nw = min(7, H - w0)
                            pt = scps_cur[0].tile([128, 512], F32, tag="sc")
                            for k in range(nw):
                                w = w0 + k
                                nc.tensor.matmul(
                                    pt[:65, k * 66:k * 66 + 66],
                                    ap(qK, 32 * hi, 8, w, [[H, H]]),
                                    ap(qQ, 32 * hi, 8, w, [[H, 66]]),
                                    start=True, stop=True,
                                    tile_position=tpos)
                            nc.scalar.activation(
                                ap(er, 0, 65, w0 * 66, [[66, nw], [1, H]]),
                                ap(pt, 0, 65, 0, [[66, nw], [1, H]]),
                                AF.Exp)
                            yield
                        # er2 (pixel-partitioned) + ertail
                        nc.scalar.dma_start(out=t["er2"][0:64, :],
                                            in_=er[0:64, :])
                        yield
                        nc.scalar.dma_start(out=t["er2"][64:128,
                                                         :66 * H - 66],
                                            in_=er[0:64, 66:])
                        yield
                        nc.scalar.dma_start(out=t["ertail"][:, :],
                                            in_=ap(er, 64, 1, 0,
                                                   [[66, H], [1, H]]))
                        yield
                        # Zr: fold (w,66)->(w,34) on Pool in 4 pieces, then
                        # DVE reduce-34 -> zravg[h,w]
                        zf1 = t["zf1"]
                        for wz0 in range(0, H, 17):
                            nwz = min(17, H - wz0)
                            pool_fold(
                                ap(zf1, 0, 65, wz0 * 34, [[34, nwz], [1, 33]]),
                                ap(er, 0, 65, wz0 * 66, [[66, nwz], [1, 33]]),
                                ap(er, 0, 65, wz0 * 66 + 33,
                                   [[66, nwz], [1, 33]]))
                            yield
                        nc.vector.tensor_reduce(
                            t["zravg"][:, :],
                            ap(zf1, 0, 65, 0, [[34, H], [1, 34]]),
                            AX.X, AluOpType.add)
                        yield
                        # c scores: per h -> psum [j, w]; exp -> ecs (f32r)
                        for h0 in range(0, H, 7):
                            nh = min(7, H - h0)
                            pt = scps_cur[0].tile([128, 512], F32, tag="sc")
                            for k in range(nh):
                                h = h0 + k
                                nc.tensor.matmul(
                                    pt[:65, k * 66:k * 66 + 66],
                                    ap(qC1, 32 * hi, 8, h * H, [[1, H]]),
                                    ap(qC2, 32 * hi, 8, h * H, [[1, 66]]),
                                    start=True, stop=True,
                                    tile_position=tpos)
                            nhs = min(nh, 64 - h0)
                            nc.scalar.activation(
                                ap(ecs, 0, 65, h0, [[1, nhs], [64, H]]),
                                ap(pt, 0, 65, 0, [[66, nhs], [1, H]]),
                                AF.Exp)
                            if h0 + nh == 65:
                                nc.scalar.activation(
                                    ectail[:, :],
                                    ap(pt, 0, 65, (nh - 1) * 66, [[1, H]]),
                                    AF.Exp)
                            yield
                        # Zc per chunk (chunk-partitioned ones-matmuls)
                        zct = scps_cur[0].tile([128, 512], F32, tag="sc")
                        for wb0 in range(0, 32, 16):
                            for wb in range(wb0, wb0 + 16):
                                nc.tensor.matmul(
                                    ap(zct, 0, 128, 2 * wb, [[1, 2]]),
                                    ap(ecs, 0, 65, wb * 128, [[1, 128]]),
                                    ones65[:, :], start=True, stop=True)
                            yield
                        nc.tensor.matmul(
                            ap(zct, 0, 64, 64, [[1, 2]]),
                            ap(ecs, 0, 65, 64 * 64, [[1, 64]]),
                            ones65[:, :], start=True, stop=True)
                        nc.tensor.matmul(
                            ap(zct, 0, 65, 66, [[1, 2]]),
                            ap(ectail, 0, 65, 0, [[1, H]]),
                            ones65[:, :], start=True, stop=True)
                        zcc = t["zcc"]
                        nc.scalar.activation(zcc[0:64, :],
                                             ap(zct, 0, 64, 0, [[2, 34]]),
                                             AF.Copy)
                        nc.scalar.activation(zcc[64:128, 0:32],
                                             ap(zct, 64, 64, 0, [[2, 32]]),
                                             AF.Copy)
                        nc.scalar.activation(zcc[64:65, 33:34],
                                             ap(zct, 64, 1, 66, [[1, 1]]),
                                             AF.Copy)
                        yield
                        # zr chunk columns + iz scalars
                        zravg, zrc = t["zravg"], t["zrc"]
                        nc.vector.tensor_copy(
                            ap(zrc, 0, 64, 0, [[1, 32]]),
                            ap(zravg, 0, 64, 0, [[2, 32]]))
                        nc.vector.tensor_copy(
                            ap(zrc, 64, 64, 0, [[1, 32]]),
                            ap(zravg, 0, 64, 1, [[2, 32]]))
                        nc.vector.tensor_copy(zrc[0:64, 32:33],
                                              zravg[0:64, 64:65])
                        nc.scalar.dma_start(out=zrc[0:65, 33:34],
                                            in_=ap(zravg, 64, 1, 0, [[1, H]]))
                        iz2 = t["iz2"]
                        nc.vector.tensor_tensor(out=iz2[:, :], in0=zrc[:, :],
                                                in1=zcc[:, :],
                                                op=AluOpType.mult)
                        nc.vector.reciprocal(iz2[:, :], iz2[:, :])
                        yield

                    def pe_p_steps():
                        """S7 pe-conv into A_sb rows 64:128 (needs v_sb)."""
                        for w0 in range(0, H, 7):
                            nw = min(7, H - w0)
                            pt = scps_cur[0].tile([128, 512], F32, tag="sc")
                            for tt in range(9):
                                ky, kx = tt // 3, tt % 3
                                rhs = ap(v_sb, 0, 128, ky * HP + kx + w0,
                                         [[1, nw], [HP, 66]])
                                nc.tensor.matmul(
                                    pt[:64, :nw * 66],
                                    Wpe[:, tt * 64:tt * 64 + 64],
                                    rhs, start=(tt == 0), stop=(tt == 8))
                            nc.scalar.activation(
                                ap(A_sb, 64, 64, w0 * H, [[1, nw * H]]),
                                ap(pt, 0, 64, 0, [[66, nw], [1, H]]),
                                AF.Identity, bias=bpe[:, :])
                            yield

                    state = {"ptt": None, "off": 0}
                    pend = []

                    def pool_fold(out, in0, in1):
                        nc.gpsimd.tensor_tensor(out=out, in0=in0, in1=in1,
                                                op=AluOpType.add)

                    def do_chunk(t, idx, lhsT_ap, er_ap, izcol, M, dst):
                        ut = ups.tile([128, ID16], F32, tag="ut")
                        for n0 in (0, 512, 1024):
                            nn = min(512, ID16 - n0)
                            nc.tensor.matmul(ut[:M, n0:n0 + nn],
                                             lhsT_ap,
                                             t["vpt"][:, n0:n0 + nn],
                                             start=True, stop=True)
                        # m is (d, i66)-strided bf16; col 65 of each d-block
                        # is pre-zeroed so the fold chain needs no tail ops
                        # E-chunks (DVE stt from PSUM) 1 in 4; fold2 on DVE
                        ph = 2 if idx % 4 == 0 else 0
                        m = mp.tile([128, 1066], BF16, tag="m")
                        if m.offset not in _padded_m:
                            _padded_m.add(m.offset)
                            nc.gpsimd.memset(
                                ap(m, 0, 128, 65, [[66, 16], [1, 1]]), 0.0)
                        if ph == 2:
                            # DVE: (ut * iz) * er straight out of PSUM (1x)
                            nc.vector.scalar_tensor_tensor(
                                out=ap(m, 0, M, 0, [[66, 16], [1, 65]]),
                                in0=ap(ut, 0, M, 0, [[65, 16], [1, 65]]),
                                scalar=t["iz2"][:M, izcol:izcol + 1],
                                in1=er_ap,
                                op0=AluOpType.mult, op1=AluOpType.mult)
                        else:
                            # Act drains PSUM to bf16 applying iz via scale;
                            # DVE multiplies by raw er at 2x
                            utb = up2.tile([128, ID16], BF16, tag="utb")
                            nc.scalar.activation(
                                ap(utb, 0, M, 0, [[65, 16], [1, 65]]),
                                ap(ut, 0, M, 0, [[65, 16], [1, 65]]),
                                AF.Identity,
                                scale=t["iz2"][:M, izcol:izcol + 1])
                            nc.vector.tensor_tensor(
                                out=ap(m, 0, M, 0, [[66, 16], [1, 65]]),
                                in0=ap(utb, 0, M, 0, [[65, 16], [1, 65]]),
                                in1=er_ap, op=AluOpType.mult)
                        # Pool fold 66 -> 34 via stt-add (col 33 pre-zeroed)
                        mh = mhp.tile([128, 544], BF16, tag="mh")
                        if mh.offset not in _padded_m:
                            _padded_m.add(mh.offset)
                            nc.gpsimd.memset(
                                ap(mh, 0, 128, 33, [[34, 16], [1, 1]]), 0.0)
                        pool_fold(ap(mh, 0, M, 0, [[34, 16], [1, 33]]),
                                  ap(m, 0, M, 0, [[66, 16], [1, 33]]),
                                  ap(m, 0, M, 33, [[66, 16], [1, 33]]))
                        # fold2+reduce deferred one chunk so DVE never
                        # stalls waiting on this chunk's Pool fold1
                        pend2.append((mh, M, dst))
                        if len(pend2) > 1:
                            do_tail(*pend2.pop(0))

                    def do_tail(mh, M, dst):
                        m2 = m2p.tile([128, 272], BF16, tag="m2")
                        nc.vector.tensor_tensor(
                            out=ap(m2, 0, M, 0, [[17, 16], [1, 17]]),
                            in0=ap(mh, 0, M, 0, [[34, 16], [1, 17]]),
                            in1=ap(mh, 0, M, 17, [[34, 16], [1, 17]]),
                            op=AluOpType.add)
                        at = tp.tile([128, 16], F32, tag="at")
                        nc.vector.tensor_reduce(
                            at[:M, :], ap(m2, 0, M, 0, [[17, 16], [1, 17]]),
                            AX.X, AluOpType.add)
                        pend.append((at, M, dst))

                    def emit_transpose(at, M):
                        if state["ptt"] is None:
                            pttt = pttp.tile([128, 512], F32, tag="ptt")
                            state["ptt"] = pttt
                            state["off"] = 0
                        nc.tensor.transpose(
                            ap(state["ptt"], 0, 16, state["off"], [[1, M]]),
                            at[:M, :], ident[:M, :M].bitcast(F32))
                        state["off"] += M

                    def drain_pend(keep):
                        while len(pend) > keep:
                            at, M, dst = pend.pop(0)
                            emit_transpose(at, M)
                            if dst is not None:
                                nc.scalar.activation(
                                    dst,
                                    ap(state["ptt"], 0, 16, 0,
                                       [[1, state["off"]]]),
                                    AF.Copy)
                                state["ptt"] = None

                    tiles = alloc_head()
                    for _ in prologue_steps(0, tiles):
                        pass
                    for hi in range(4):
                        A0 = 16 * hi
                        t = tiles
                        if hi < 3:
                            tiles = alloc_head()
                            nxt = prologue_steps(hi + 1, tiles)
                        else:
                            nxt = pe_p_steps()
                        er, ertail = t["er"], t["ertail"]
                        for wb in range(32):
                            dst = (ap(A_sb, A0, 16, 2 * (wb - 3) * H,
                                      [[H, 8], [1, 64]])
                                   if wb % 4 == 3 else None)
                            do_chunk(t, wb,
                                     ap(t["ecs"], 0, 65, wb * 128, [[1, 128]]),
                                     ap(t["er2"], 0, 128, 2 * wb * 66,
                                        [[0, 16], [1, H]]),
                                     wb, 128, dst)
                            drain_pend(8)
                            next(nxt, None)
                        do_chunk(t, 32,
                                 ap(t["ecs"], 0, 65, 64 * 64, [[1, 64]]),
                                 ap(t["er2"], 0, 64, 64 * 66,
                                    [[0, 16], [1, H]]),
                                 32, 64, ap(A_sb, A0, 16, 64 * H, [[1, 64]]))
                        drain_pend(8)
                        next(nxt, None)
                        do_chunk(t, 33,
                                 ap(t["ectail"], 0, 65, 0, [[1, H]]),
                                 ap(ertail, 0, 65, 0, [[0, 16], [1, H]]),
                                 33, 65, ap(A_sb, A0, 16, 64, [[H, H]]))
                        drain_pend(8)
                        for _ in nxt:
                            pass
                    drain_pend(0)

            # ===== S8: dconv(A) + dconv(P) -> dc3 rows 0:32 =====
            with (
                tc.tile_pool(name="dcpool", bufs=1) as dcp,
                tc.tile_pool(name="ps8", bufs=2, space="PSUM") as ps8,
            ):
                dc3 = dcp.tile([96, IMGP * IMGP], BF16, tag="dc3")
                # zero borders: block b holds dcpad rows shifted by b, so
                # block0 rows {0,129}, block1 rows {128,129-ish}, block2
                # rows {127,128}; plus the 1-px column strips everywhere.
                nc.gpsimd.dma_start(out=ap(dc3, 0, 32, 0, [[1, IMGP]]),
                                    in_=zbf[:32, :IMGP])
                nc.sync.dma_start(
                    out=ap(dc3, 0, 32, 129 * IMGP, [[1, IMGP]]),
                    in_=zbf[:32, :IMGP])
                nc.sync.dma_start(
                    out=ap(dc3, 32, 32, 128 * IMGP, [[1, 2 * IMGP]]),
                    in_=zbf[:32, :2 * IMGP])
                nc.sync.dma_start(
                    out=ap(dc3, 64, 32, 127 * IMGP, [[1, 3 * IMGP]]),
                    in_=zbf[:32, :3 * IMGP])
                for blk in range(3):
                    nc.sync.dma_start(
                        out=ap(dc3, 32 * blk, 32, IMGP, [[IMGP, 128], [1, 1]]),
                        in_=zbf[:32, :128])
                    nc.sync.dma_start(
                        out=ap(dc3, 32 * blk, 32, IMGP + 129,
                               [[IMGP, 128], [1, 1]]),
                        in_=zbf[:32, :128])
                # a0-outer so dc rows complete in ascending order; the
                # row-shifted copies for the 96-deep final conv are issued
                # piecewise so S9 can pipeline behind S8.
                shift_done = 0

                def dc3_shift_upto(row):
                    nonlocal shift_done
                    lo = shift_done
                    if row <= lo:
                        return
                    nc.sync.dma_start(
                        out=dc3[32:64, lo * IMGP:row * IMGP],
                        in_=dc3[0:32, (lo + 1) * IMGP:(row + 1) * IMGP])
                    nc.gpsimd.dma_start(
                        out=dc3[64:96, lo * IMGP:row * IMGP],
                        in_=dc3[0:32, (lo + 2) * IMGP:(row + 2) * IMGP])
                    shift_done = row

                for a0 in range(0, 64, 8):
                    for pr in range(2):
                        for ps in range(2):
                            pt = ps8.tile([32, 512], F32, tag="dcps")
                            w0 = (pr * 2 + ps) * 32
                            nc.tensor.matmul(
                                pt[:, :], Wdc[:, w0:w0 + 32],
                                ap(A_sb, 0, 128, ps * H + pr + a0,
                                   [[1, 8], [H, 64]]),
                                start=True, stop=True)
                            dst = ap(dc3, 0, 32,
                                     (2 * a0 + pr + 1) * IMGP + ps + 1,
                                     [[2 * IMGP, 8], [2, 64]])
                            if (pr * 2 + ps) % 2 == 0:
                                nc.scalar.activation(dst, pt[:, :],
                                                     AF.Identity,
                                                     bias=bdc[:, :])
                            else:
                                nc.vector.scalar_tensor_tensor(
                                    out=dst, in0=pt[:, :], scalar=1.0,
                                    in1=ap(bdc, 0, 32, 0, [[0, 8], [0, 64]]),
                                    op0=AluOpType.mult, op1=AluOpType.add)
                    if a0 in (24, 40, 56):
                        # rows complete up to 2*a0+16 after this block
                        dc3_shift_upto(2 * a0 + 14)
                nc.sync.dma_start(
                    out=dc3[32:64, shift_done * IMGP:IMGP * IMGP - IMGP],
                    in_=dc3[0:32, (shift_done + 1) * IMGP:])
                nc.gpsimd.dma_start(
                    out=dc3[64:96, shift_done * IMGP:IMGP * IMGP - 2 * IMGP],
                    in_=dc3[0:32, (shift_done + 2) * IMGP:IMGP * IMGP])

                # ===== S9: final conv partial, 96-deep =====
                with (
                    tc.tile_pool(name="opool", bufs=2) as op_,
                    tc.tile_pool(name="ps9", bufs=2, space="PSUM") as ps9,
                ):
                    ost = None
                    for r0 in range(0, IMG, 4):
                        pt = ps9.tile([64, 512], F32, tag="o")
                        for kx in range(3):
                            rhs = ap(dc3, 0, 96, r0 * IMGP + kx,
                                     [[IMGP, 4], [1, IMG]])
                            nc.tensor.matmul(pt[:, :],
                                             Wout3[:, kx * 64:kx * 64 + 64],
                                             rhs, start=(kx == 0),
                                             stop=(kx == 2))
                        if r0 % 16 == 0:
                            ost = op_.tile([64, 2048], BF16, tag="ost")
                        sl = (r0 % 16) // 4 * 512
                        if (r0 // 4) % 2 == 0:
                            nc.scalar.activation(ost[:, sl:sl + 512], pt[:, :],
                                                 AF.Identity, bias=bfin[:, :])
                        else:
                            nc.vector.scalar_tensor_tensor(
                                out=ost[:, sl:sl + 512], in0=pt[:, :],
                                scalar=1.0,
                                in1=ap(bfin, 0, 64, 0, [[0, 512]]),
                                op0=AluOpType.mult, op1=AluOpType.add)
                        if r0 % 16 == 12:
                            oeng = nc.sync if r0 % 32 == 12 else nc.gpsimd
                            oeng.dma_start(out=out_d[:, r0 - 12:r0 + 4, :],
                                           in_=ost[:, :])

    nc.compile()
    return nc


_NC_CACHE = None


def kernel(**inputs):
    global _NC_CACHE
    if _NC_CACHE is None:
        _NC_CACHE = build_nc()
    nc = _NC_CACHE
    in_maps = [prep_core_inputs(inputs, c // 2, c % 2) for c in range(8)]
    res = run_bass_kernel_spmd(nc, in_maps, list(range(8)))
    out = np.zeros((4, 64, IMG, IMG), np.float32)
    for b in range(4):
        out[b] = (res.results[2 * b]["out"].astype(np.float32) +
                  res.results[2 * b + 1]["out"].astype(np.float32))
    return out



# revision 45
# speedup vs baseline: 1.1879x; 1.0023x over previous
"""Trainium2 Bass kernel for nn_MatrixAttention (sparse_attention).

Sharding: 8 cores = (batch b in 0..3) x (head-group g in 0..1, 4 heads each).
Each core: in_proj -> rcv conv (its 192 ch) -> row/col attention (4 heads)
-> pe conv -> grouped deconv (its 32 dc ch) -> partial final 3x3 conv over
all 64 output channels from its 32 dc channels. Host gather sums the pair
partials (input-dim-sharded conv => reduce-gather) and stacks batches.

Perf structure (TimelineSim-tuned):
- q/ecs/vpt/scores all bf16 (f32r matmuls with <256-col outputs pay a 4x
  cycle penalty; bf16 is 1 cycle/row and halves SBUF).
- S2 split: G1 (q) chunks first, then one merged q-reshuffle (sync+gpsimd
  DGE queues; the ~630ns/DMA descriptor-gen serializes on HWDGE, so it
  must never sit on the Act/SP queues mid-pipeline), then G2 (v) chunks
  with the head-0 prologue generator interleaved (scores/exp/Zc/Zr/iz
  hide behind G2's PE work; V-permute last since it needs v).
- Combine chunks (128 px): PE matmul (ec^T V, 65-deep, 1040 cols) ->
  D-chunks: Act drain (iz scale) to bf16 + DVE tensor_tensor x er (2x) |
  E-chunks (1 in 4): DVE stt from PSUM (1x) -> Pool fold 65->33 ->
  DVE reduce-33 -> PE transpose -> Act flush into A.
  The fold2/reduce of chunk N runs at the TOP of chunk N+2 (pend2) and
  transposes/flushes are deferred 4 chunks (pend): in-order engine queues
  otherwise serialize the whole chain per chunk.
- Zr via Pool fold (w,66-stride er, pad col zeroed per head) + DVE
  reduce-33; Zc via ones-matmuls; iz=1/(Zr*Zc) as drain scale/stt scalar.
- S8/S9 drains alternate Act / DVE-stt(+bias broadcast); output stores
  batched 4 row-groups per DMA (HWDGE descriptor-gen is the tail limit).

Self-contained: hardcodes all shapes; no sibling imports.
"""
import sys
import numpy as np

sys.path.insert(0, "/opt/trn_rl_repo")

import ml_dtypes                        # noqa: E402
import concourse.bass as bass           # noqa: E402
import concourse.bacc as bacc           # noqa: E402
import concourse.mybir as mybir         # noqa: E402
from concourse.tile import TileContext  # noqa: E402
from concourse.bass_utils import run_bass_kernel_spmd  # noqa: E402
from concourse.alu_op_type import AluOpType  # noqa: E402

F32 = mybir.dt.float32
F32R = mybir.dt.float32r
BF16 = mybir.dt.bfloat16
AF = mybir.ActivationFunctionType
AX = mybir.AxisListType
BF16NP = ml_dtypes.bfloat16

NH, KD, HD = 8, 8, 16
SCALE = KD ** -0.5
H = 65            # spatial after in_proj
HP = 67           # padded
NPIX = H * H      # 4225
PADPIX = HP * HP  # 4489
IMG = 128
IMGP = 130
ID16 = 1040       # (i,d) = 65*16
NECS = 65 * 64    # 4160: w-major (h<64) ec storage


def r32(x):
    return x.bitcast(F32R)


def ap(tile, part0, nparts, free_off, free_dims):
    """AP over a tile: partitions [part0, part0+nparts), free offset + dims
    (list of [step, count], outer->inner)."""
    pitch = tile.ap[0][0]
    return bass.AP(tile.tensor, tile.offset + part0 * pitch + free_off,
                   [[pitch, nparts]] + [list(d) for d in free_dims])


# ----------------------------------------------------------------------------
# Host-side weight prep
# ----------------------------------------------------------------------------
def prep_core_inputs(inputs, b, g):
    inp = {k: np.ascontiguousarray(np.asarray(v), dtype=np.float32)
           for k, v in inputs.items()}
    heads = list(range(4 * g, 4 * g + 4))

    xp = np.zeros((64, IMGP, IMGP), np.float32)
    xp[:, 1:129, 1:129] = inp["x"][b]
    xp = xp.reshape(64, IMGP * IMGP)

    W1 = np.zeros((2, 2, 64, 128), np.float32)
    for co in range(128):
        W1[:, :, co // 2, co] = inp["w_in"][co, 0] * inp["s_in"][co]
    W1 = W1.reshape(4, 64, 128).transpose(1, 0, 2).reshape(64, 512)
    b1 = inp["b_in"].reshape(128, 1)

    # rcv conv weights. G1 (compact q): cols = [rq 4hx8 | rk | cq | ck].
    # G2 (v, padded): col 32*hi + dd  holds v-channel dd of head hi.
    w_rcv = inp["w_rcv"] * inp["s_rcv"][:, None, None, None]
    qrows = []
    for blk in range(4):           # rq, rk, cq, ck
        for h in heads:
            qrows.extend(range(h * 48 + blk * 8, h * 48 + blk * 8 + 8))
    Wq = w_rcv[qrows]              # [128, 128, 3, 3]
    bq = inp["b_rcv"][qrows].copy()
    scale_mask = np.ones(128, np.float32)
    scale_mask[0:32] = SCALE       # rq
    scale_mask[64:96] = SCALE      # cq
    Wq = Wq * scale_mask[:, None, None, None]
    bq = bq * scale_mask
    Wv = np.zeros((128, 128, 3, 3), np.float32)   # padded v rows
    bv = np.zeros((128, 1), np.float32)
    for hi, h in enumerate(heads):
        for dd in range(16):
            Wv[32 * hi + dd] = w_rcv[h * 48 + 32 + dd]
            bv[32 * hi + dd, 0] = inp["b_rcv"][h * 48 + 32 + dd]
    # lhsT [ci=128, 9 taps, 256 cols (G1 128 | G2 128)]
    Wrcv = np.concatenate(
        [Wq.transpose(1, 2, 3, 0).reshape(128, 9, 128),
         Wv.transpose(1, 2, 3, 0).reshape(128, 9, 128)], axis=2
    ).reshape(128, 9 * 256)
    brcv_g1 = bq.reshape(128, 1)
    brcv_g2 = bv

    # pe conv: input/output both padded to 128 (head hi at rows/cols 32*hi)
    w_pe = inp["w_pe"] * inp["s_pe"][:, None, None, None]
    Wpe = np.zeros((128, 3, 3, 64), np.float32)
    bpe = np.zeros((64, 1), np.float32)
    for hi, h_abs in enumerate(heads):
        for col in range(16):
            co = h_abs * 16 + col
            col_l = 16 * hi + col
            for k in range(2):
                ci_row = 32 * hi + 2 * (col // 2) + k
                Wpe[ci_row, :, :, col_l] = w_pe[co, k]
            bpe[col_l, 0] = inp["b_pe"][co]
    Wpe = Wpe.reshape(128, 9 * 64)

    w_dc = inp["w_dc"]
    g0 = heads[0] * 8
    # rows: A-compact channels 0:64, P-compact channels 64:128 (dconv of
    # A+P done as one 128-deep matmul over the merged AP tile)
    Wdc = np.zeros((128, 2, 2, 32), np.float32)
    bdc = np.zeros((32, 1), np.float32)
    for cl in range(32):
        co = g0 + cl
        hi, c = cl // 8, cl % 8
        for k in range(2):
            Wdc[16 * hi + 2 * c + k, :, :, cl] = w_dc[co, k]
            Wdc[64 + 16 * hi + 2 * c + k, :, :, cl] = w_dc[co, k]
        bdc[cl, 0] = inp["b_dc"][co]
    Wdc = Wdc.reshape(128, 4 * 32)

    # final conv, 96-deep (ky folded into contraction): rows (ky, ci32),
    # cols (kx, co64)
    w_out = inp["w_out"] * inp["s_out"][:, None, None, None]   # [64,64,3,3]
    Wout3 = np.zeros((96, 3, 64), np.float32)
    for ky in range(3):
        for ci in range(32):
            for kx in range(3):
                Wout3[ky * 32 + ci, kx, :] = w_out[:, 32 * g + ci, ky, kx]
    Wout3 = Wout3.reshape(96, 192)
    bfin = (inp["b_out"] if g == 0 else np.zeros(64, np.float32)).reshape(64, 1)

    return {
        "xp": xp.astype(BF16NP), "W1": np.ascontiguousarray(W1).astype(BF16NP), "b1": b1,
        "Wrcv": np.ascontiguousarray(Wrcv).astype(BF16NP),
        "brcv_g1": brcv_g1, "brcv_g2": brcv_g2,
        "Wpe": np.ascontiguousarray(Wpe), "bpe": bpe,
        "Wdc": np.ascontiguousarray(Wdc).astype(BF16NP), "bdc": bdc,
        "Wout3": np.ascontiguousarray(Wout3).astype(BF16NP), "bfin": bfin,
        "ident": np.eye(128, dtype=np.float32),
        "ones": np.ones((65, 2), np.float32).astype(BF16NP),
        "zeros": np.zeros((128, PADPIX), np.float32),
    }


# ----------------------------------------------------------------------------
# Device program
# ----------------------------------------------------------------------------
def build_nc():
    nc = bacc.Bacc(None, target_bir_lowering=False)

    dins = {}
    for name, shape, dt_ in [
        ("xp", [64, IMGP * IMGP], BF16), ("W1", [64, 512], BF16),
        ("b1", [128, 1], F32),
        ("Wrcv", [128, 2304], BF16), ("brcv_g1", [128, 1], F32),
        ("brcv_g2", [128, 1], F32),
        ("Wpe", [128, 576], F32R), ("bpe", [64, 1], F32),
        ("Wdc", [128, 128], BF16), ("bdc", [32, 1], F32),
        ("Wout3", [96, 192], BF16), ("bfin", [64, 1], F32),
        ("ident", [128, 128], F32R),
        ("ones", [65, 2], BF16),
        ("zeros", [128, PADPIX], F32R),
    ]:
        dins[name] = nc.dram_tensor(name, shape, dt_, kind="ExternalInput")
    out_d = nc.dram_tensor("out", [64, IMG, IMG], BF16, kind="ExternalOutput")
    zbf = dins["zeros"].bitcast(BF16)   # [128, 2*PADPIX] of bf16 zeros

    with TileContext(nc) as tc:
        with (
            tc.tile_pool(name="wpool", bufs=1) as wp,
            tc.tile_pool(name="vpool", bufs=1) as vp_,
            tc.tile_pool(name="apool", bufs=1) as ap_,
        ):
            def load(name, shape, dt_=F32):
                t = wp.tile(shape, dt_, tag=name)
                # weights go on the Pool SWDGE queue so the x/W1 loads on
                # the SP/Act HWDGE queues start immediately
                eng = nc.gpsimd if shape[0] * shape[1] > 4096 else nc.sync
                eng.dma_start(out=t[:, :], in_=dins[name][:, :])
                return t

            Wrcv = load("Wrcv", [128, 2304], BF16)
            brg1 = load("brcv_g1", [128, 1])
            brg2 = load("brcv_g2", [128, 1])
            Wpe = load("Wpe", [128, 576], F32R)
            bpe = load("bpe", [64, 1])
            Wdc = load("Wdc", [128, 128], BF16)
            bdc = load("bdc", [32, 1])
            Wout3 = load("Wout3", [96, 192], BF16)
            bfin = load("bfin", [64, 1])
            ident = load("ident", [128, 128], F32R)
            ones65 = load("ones", [65, 2], BF16)

            v_sb = vp_.tile([128, PADPIX + 2 * HP], F32R, tag="v")  # (h,w) pad
            nc.vector.memset(v_sb[:, :].bitcast(F32), 0.0)
            # merged tile: rows 0:64 = attention A (16 per head, compact),
            # rows 64:128 = pe-conv P (compact); no pad rows
            A_sb = ap_.tile([128, NPIX], BF16, tag="A")      # (w,h)-major

            with tc.tile_pool(name="qxpool", bufs=1) as qx:
                qQ = qx.tile([128, NPIX + H], BF16, tag="qQ")
                qK = qx.tile([128, NPIX + H], BF16, tag="qK")
                qC1 = qx.tile([128, NPIX + H], BF16, tag="qC1")
                qC2 = qx.tile([128, NPIX + H], BF16, tag="qC2")
                for _t in (qQ, qK, qC1, qC2):
                    nc.gpsimd.memset(_t[:, NPIX:], 0.0)

                with tc.tile_pool(name="ypool", bufs=1) as yp:
                    y_sb = yp.tile([128, PADPIX + 2 * HP + 1], BF16, tag="y")
                    nc.gpsimd.memset(y_sb[:, :].bitcast(F32), 0.0)

                    # ===== S1: in_proj (x loaded in two halves) =====
                    with (
                        tc.tile_pool(name="xpool", bufs=2) as xp_pool,
                        tc.tile_pool(name="ps1", bufs=2, space="PSUM") as ps1,
                    ):
                        W1 = xp_pool.tile([64, 512], BF16, tag="w1")
                        nc.sync.dma_start(out=W1[:, :], in_=dins["W1"][:, :])
                        b1 = xp_pool.tile([128, 1], F32, tag="b1")
                        nc.sync.dma_start(out=b1[:, :], in_=dins["b1"][:, :])

                        chunks = [(0, 7), (7, 7), (14, 7), (21, 7), (28, 4),
                                  (32, 7), (39, 7), (46, 7), (53, 7), (60, 5)]
                        for half in range(2):
                            xt = xp_pool.tile([64, 68 * IMGP], BF16, tag="x")
                            src_off = 0 if half == 0 else 64 * IMGP
                            nc.sync.dma_start(
                                out=xt[:, :33 * IMGP],
                                in_=dins["xp"][:, src_off:src_off + 33 * IMGP])
                            nc.sync.dma_start(
                                out=xt[:, 33 * IMGP:66 * IMGP],
                                in_=dins["xp"][:, src_off + 33 * IMGP:
                                               src_off + 66 * IMGP])
                            nc.gpsimd.memset(xt[:, 66 * IMGP:].bitcast(F32), 0.0)
                            row0 = 0 if half == 0 else 64
                            for c0, nr in chunks:
                                if (half == 0) != (c0 < 32):
                                    continue
                                pt = ps1.tile([128, 7 * 66], F32, tag="ps1")
                                for t, (ky, kx) in enumerate(
                                        [(0, 0), (0, 1), (1, 0), (1, 1)]):
                                    rhs = ap(xt, 0, 64,
                                             (2 * c0 + ky - row0) * IMGP + kx,
                                             [[2 * IMGP, nr], [2, 66]])
                                    nc.tensor.matmul(
                                        pt[:, :nr * 66],
                                        W1[:, t * 128:(t + 1) * 128],
                                        rhs, start=(t == 0), stop=(t == 3))
                                dst = ap(y_sb, 0, 128, (c0 + 1) * HP + 1,
                                         [[HP, nr], [1, H]])
                                nc.scalar.activation(dst,
                                                     ap(pt, 0, 128, 0,
                                                        [[66, nr], [1, H]]),
                                                     AF.Identity, bias=b1[:, :])

                    # ===== S2: rcv conv (q compact bf16 + v padded f32r) ====
                    with (
                        tc.tile_pool(name="qcpool", bufs=1) as qcp,
                        tc.tile_pool(name="ps2", bufs=2, space="PSUM") as ps2,
                    ):
                        q_sb = qcp.tile([128, NPIX], BF16, tag="qc")
                        for c0 in range(0, H, 7):
                            nr = min(7, H - c0)
                            pt = ps2.tile([128, 7 * 66], F32, tag="ps2")
                            for t in range(9):
                                ky, kx = t // 3, t % 3
                                rhs = ap(y_sb, 0, 128, (c0 + ky) * HP + kx,
                                         [[HP, nr], [1, 66]])
                                nc.tensor.matmul(
                                    pt[:, :nr * 66],
                                    Wrcv[:, t * 256:t * 256 + 128],
                                    rhs, start=(t == 0), stop=(t == 8))
                            nc.scalar.activation(q_sb[:, c0 * H:(c0 + nr) * H],
                                                 ap(pt, 0, 128, 0,
                                                    [[66, nr], [1, H]]),
                                                 AF.Identity, bias=brg1[:, :])
                            pt2 = ps2.tile([128, 7 * 66], F32, tag="ps2")
                            for t in range(9):
                                ky, kx = t // 3, t % 3
                                rhs = ap(y_sb, 0, 128, (c0 + ky) * HP + kx,
                                         [[HP, nr], [1, 66]])
                                nc.tensor.matmul(
                                    pt2[:, :nr * 66],
                                    Wrcv[:, t * 256 + 128:t * 256 + 256],
                                    rhs, start=(t == 0), stop=(t == 8))
                            dstv = ap(v_sb, 0, 128, (c0 + 1) * HP + 1,
                                      [[HP, nr], [1, H]])
                            nc.scalar.activation(dstv,
                                                 ap(pt2, 0, 128, 0,
                                                    [[66, nr], [1, H]]),
                                                 AF.Identity, bias=brg2[:, :])
                        # reshuffle q -> 32-aligned padded tensors (sbuf
                        # dma, spread across DGE queues to parallelize issue)
                        qeng = [nc.sync, nc.scalar]
                        for hi in range(4):
                            for blk, dstq in enumerate([qQ, qK, qC1, qC2]):
                                qeng[(hi * 4 + blk) % 2].dma_start(
                                    out=ap(dstq, 32 * hi, 8, 0, [[1, NPIX]]),
                                    in_=q_sb[blk * 32 + 8 * hi:
                                             blk * 32 + 8 * hi + 8, :])

                # ===== S3-S6: attention, software-pipelined per head =====
                # Per-head prologue (scores/exp/Z/iz/V-permute) is emitted as
                # generator steps interleaved into the PREVIOUS head's chunk
                # loop, so PE-heavy score work overlaps DVE/Pool-heavy chunks.
                with (
                    tc.tile_pool(name="hpool", bufs=2) as hp,
                    tc.tile_pool(name="mpool", bufs=5) as mp,
                    tc.tile_pool(name="up2pool", bufs=4) as up2,
                    tc.tile_pool(name="mhpool", bufs=7) as mhp,
                    tc.tile_pool(name="m2pool", bufs=2) as m2p,
                    tc.tile_pool(name="tpool", bufs=14) as tp,
                    tc.tile_pool(name="scps", bufs=1, space="PSUM") as scps,
                    tc.tile_pool(name="ups", bufs=2, space="PSUM") as ups,
                    tc.tile_pool(name="pttps", bufs=1, space="PSUM") as pttp,
                ):
                    _padded_heads = set()
                    _padded_m = set()

                    def alloc_head():
                        t = {}
                        for nm, shape, dt_ in [
                            ("er", [65, 66 * H], BF16),      # (w, i) 66-stride
                            ("er2", [128, 66 * H], BF16),
                            ("ertail", [65, 65], BF16),
                            ("ecs", [65, NECS], BF16),
                            ("ectail", [65, 65], BF16),
                            ("zf1", [65, 34 * H], BF16),     # Zr fold1 (w,34)
                            ("zravg", [65, 65], F32),
                            ("zrc", [128, 34], F32),
                            ("zcc", [128, 34], F32),
                            ("iz2", [128, 34], F32),
                            ("vpt", [65, ID16], BF16),
                        ]:
                            tl = hp.tile(shape, dt_, tag=nm)
                            t[nm] = tl
                        # zero pad columns read by the fold chains (col 65
                        # of each er 66-block; col 33 of zf1; col 17 of zf2)
                        if t["er"].offset not in _padded_heads:
                            _padded_heads.add(t["er"].offset)
                            nc.gpsimd.memset(
                                ap(t["er"], 0, 65, 65, [[66, H], [1, 1]]), 0.0)
                            nc.gpsimd.memset(
                                ap(t["zf1"], 0, 65, 33, [[34, H], [1, 1]]), 0.0)
                        return t

                    def prologue_steps(hi, t):
                        """Generator: emits one instruction group per next()."""
                        tpos = (32 * hi, 0)
                        er, ecs, ectail = t["er"], t["ecs"], t["ectail"]
                        # V-permute first (only needs v_sb)
                        for i0 in range(0, H, 32):
                            ni = min(32, H - i0)
                            ptv = scps.tile([128, 512], F32, tag="sc")
                            for k in range(ni):
                                i = i0 + k
                                src = ap(v_sb, 32 * hi, 16,
                                         (i + 1) * HP + 1, [[1, H]])
                                idn = ap(ident, 32 * hi, 16, 32 * hi,
                                         [[1, 16]])
                                nc.tensor.transpose(
                                    r32(ap(ptv, 0, 65, k * 16, [[1, 16]])),
                                    src, idn, tile_position=tpos)
                            nc.scalar.activation(
                                ap(t["vpt"], 0, 65, i0, [[1, ni], [H, 16]]),
                                ptv[:65, :ni * 16], AF.Copy)
                            yield
                        # r scores: per w -> psum [h, i]; exp -> er (bf16,
                        # 66-stride per w so the Zr fold chain needs no tail)
                        for w0 in range(0, H, 7):
                            nw = min(7, H - w0)
                            pt = scps_cur[0].tile([128, 512], F32, tag="sc")
                            for k in range(nw):
                                w = w0 + k
                                nc.tensor.matmul(
                                    pt[:65, k * 66:k * 66 + 66],
                                    ap(qK, 32 * hi, 8, w, [[H, H]]),
                                    ap(qQ, 32 * hi, 8, w, [[H, 66]]),
                                    start=True, stop=True,
                                    tile_position=tpos)
                            nc.scalar.activation(
                                ap(er, 0, 65, w0 * 66, [[66, nw], [1, H]]),
                                ap(pt, 0, 65, 0, [[66, nw], [1, H]]),
                                AF.Exp)
                            yield
                        # er2 (pixel-partitioned) + ertail
                        nc.scalar.dma_start(out=t["er2"][0:64, :],
                                            in_=er[0:64, :])
                        yield
                        nc.scalar.dma_start(out=t["er2"][64:128,
                                                         :66 * H - 66],
                                            in_=er[0:64, 66:])
                        yield
                        nc.scalar.dma_start(out=t["ertail"][:, :],
                                            in_=ap(er, 64, 1, 0,
                                                   [[66, H], [1, H]]))
                        yield
                        # Zr: fold (w,66)->(w,34) on Pool in 4 pieces, then
                        # DVE reduce-34 -> zravg[h,w]
                        zf1 = t["zf1"]
                        for wz0 in range(0, H, 17):
                            nwz = min(17, H - wz0)
                            pool_fold(
                                ap(zf1, 0, 65, wz0 * 34, [[34, nwz], [1, 33]]),
                                ap(er, 0, 65, wz0 * 66, [[66, nwz], [1, 33]]),
                                ap(er, 0, 65, wz0 * 66 + 33,
                                   [[66, nwz], [1, 33]]))
                            yield
                        nc.vector.tensor_reduce(
                            t["zravg"][:, :],
                            ap(zf1, 0, 65, 0, [[34, H], [1, 34]]),
                            AX.X, AluOpType.add)
                        yield
                        # c scores: per h -> psum [j, w]; exp -> ecs (f32r)
                        for h0 in range(0, H, 7):
                            nh = min(7, H - h0)
                            pt = scps_cur[0].tile([128, 512], F32, tag="sc")
                            for k in range(nh):
                                h = h0 + k
                                nc.tensor.matmul(
                                    pt[:65, k * 66:k * 66 + 66],
                                    ap(qC1, 32 * hi, 8, h * H, [[1, H]]),
                                    ap(qC2, 32 * hi, 8, h * H, [[1, 66]]),
                                    start=True, stop=True,
                                    tile_position=tpos)
                            nhs = min(nh, 64 - h0)
                            nc.scalar.activation(
                                ap(ecs, 0, 65, h0, [[1, nhs], [64, H]]),
                                ap(pt, 0, 65, 0, [[66, nhs], [1, H]]),
                                AF.Exp)
                            if h0 + nh == 65:
                                nc.scalar.activation(
                                    ectail[:, :],
                                    ap(pt, 0, 65, (nh - 1) * 66, [[1, H]]),
                                    AF.Exp)
                            yield
                        # Zc per chunk (chunk-partitioned ones-matmuls)
                        zct = scps_cur[0].tile([128, 512], F32, tag="sc")
                        for wb0 in range(0, 32, 16):
                            for wb in range(wb0, wb0 + 16):
                                nc.tensor.matmul(
                                    ap(zct, 0, 128, 2 * wb, [[1, 2]]),
                                    ap(ecs, 0, 65, wb * 128, [[1, 128]]),
                                    ones65[:, :], start=True, stop=True)
                            yield
                        nc.tensor.matmul(
                            ap(zct, 0, 64, 64, [[1, 2]]),
                            ap(ecs, 0, 65, 64 * 64, [[1, 64]]),
                            ones65[:, :], start=True, stop=True)
                        nc.tensor.matmul(
                            ap(zct, 0, 65, 66, [[1, 2]]),
                            ap(ectail, 0, 65, 0, [[1, H]]),
                            ones65[:, :], start=True, stop=True)
                        zcc = t["zcc"]
                        nc.scalar.activation(zcc[0:64, :],
                                             ap(zct, 0, 64, 0, [[2, 34]]),
                                             AF.Copy)
                        nc.scalar.activation(zcc[64:128, 0:32],
                                             ap(zct, 64, 64, 0, [[2, 32]]),
                                             AF.Copy)
                        nc.scalar.activation(zcc[64:65, 33:34],
                                             ap(zct, 64, 1, 66, [[1, 1]]),
                                             AF.Copy)
                        yield
                        # zr chunk columns + iz scalars
                        zravg, zrc = t["zravg"], t["zrc"]
                        nc.vector.tensor_copy(
                            ap(zrc, 0, 64, 0, [[1, 32]]),
                            ap(zravg, 0, 64, 0, [[2, 32]]))
                        nc.vector.tensor_copy(
                            ap(zrc, 64, 64, 0, [[1, 32]]),
                            ap(zravg, 0, 64, 1, [[2, 32]]))
                        nc.vector.tensor_copy(zrc[0:64, 32:33],
                                              zravg[0:64, 64:65])
                        nc.scalar.dma_start(out=zrc[0:65, 33:34],
                                            in_=ap(zravg, 64, 1, 0, [[1, H]]))
                        iz2 = t["iz2"]
                        nc.vector.tensor_tensor(out=iz2[:, :], in0=zrc[:, :],
                                                in1=zcc[:, :],
                                                op=AluOpType.mult)
                        nc.vector.reciprocal(iz2[:, :], iz2[:, :])
                        yield

                    def pe_p_steps():
                        """S7 pe-conv into A_sb rows 64:128 (needs v_sb)."""
                        for w0 in range(0, H, 7):
                            nw = min(7, H - w0)
                            pt = scps_cur[0].tile([128, 512], F32, tag="sc")
                            for tt in range(9):
                                ky, kx = tt // 3, tt % 3
                                rhs = ap(v_sb, 0, 128, ky * HP + kx + w0,
                                         [[1, nw], [HP, 66]])
                                nc.tensor.matmul(
                                    pt[:64, :nw * 66],
                                    Wpe[:, tt * 64:tt * 64 + 64],
                                    rhs, start=(tt == 0), stop=(tt == 8))
                            nc.scalar.activation(
                                ap(A_sb, 64, 64, w0 * H, [[1, nw * H]]),
                                ap(pt, 0, 64, 0, [[66, nw], [1, H]]),
                                AF.Identity, bias=bpe[:, :])
                            yield

                    state = {"ptt": None, "off": 0}
                    pend = []

                    def pool_fold(out, in0, in1):
                        nc.gpsimd.tensor_tensor(out=out, in0=in0, in1=in1,
                                                op=AluOpType.add)

                    def do_chunk(t, idx, lhsT_ap, er_ap, izcol, M, dst):
                        ut = ups.tile([128, ID16], F32, tag="ut")
                        for n0 in (0, 512, 1024):
                            nn = min(512, ID16 - n0)
                            nc.tensor.matmul(ut[:M, n0:n0 + nn],
                                             lhsT_ap,
                                             t["vpt"][:, n0:n0 + nn],
                                             start=True, stop=True)
                        # m is (d, i66)-strided bf16; col 65 of each d-block
                        # is pre-zeroed so the fold chain needs no tail ops
                        # E-chunks (DVE stt from PSUM) 1 in 4; fold2 on DVE
                        ph = 2 if idx % 4 == 0 else 0
                        m = mp.tile([128, 1066], BF16, tag="m")
                        if m.offset not in _padded_m:
                            _padded_m.add(m.offset)
                            nc.gpsimd.memset(
                                ap(m, 0, 128, 65, [[66, 16], [1, 1]]), 0.0)
                        if ph == 2:
                            # DVE: (ut * iz) * er straight out of PSUM (1x)
                            nc.vector.scalar_tensor_tensor(
                                out=ap(m, 0, M, 0, [[66, 16], [1, 65]]),
                                in0=ap(ut, 0, M, 0, [[65, 16], [1, 65]]),
                                scalar=t["iz2"][:M, izcol:izcol + 1],
                                in1=er_ap,
                                op0=AluOpType.mult, op1=AluOpType.mult)
                        else:
                            # Act drains PSUM to bf16 applying iz via scale;
                            # DVE multiplies by raw er at 2x
                            utb = up2.tile([128, ID16], BF16, tag="utb")
                            nc.scalar.activation(
                                ap(utb, 0, M, 0, [[65, 16], [1, 65]]),
                                ap(ut, 0, M, 0, [[65, 16], [1, 65]]),
                                AF.Identity,
                                scale=t["iz2"][:M, izcol:izcol + 1])
                            nc.vector.tensor_tensor(
                                out=ap(m, 0, M, 0, [[66, 16], [1, 65]]),
                                in0=ap(utb, 0, M, 0, [[65, 16], [1, 65]]),
                                in1=er_ap, op=AluOpType.mult)
                        # Pool fold 66 -> 34 via stt-add (col 33 pre-zeroed)
                        mh = mhp.tile([128, 544], BF16, tag="mh")
                        if mh.offset not in _padded_m:
                            _padded_m.add(mh.offset)
                            nc.gpsimd.memset(
                                ap(mh, 0, 128, 33, [[34, 16], [1, 1]]), 0.0)
                        pool_fold(ap(mh, 0, M, 0, [[34, 16], [1, 33]]),
                                  ap(m, 0, M, 0, [[66, 16], [1, 33]]),
                                  ap(m, 0, M, 33, [[66, 16], [1, 33]]))
                        # fold2+reduce deferred one chunk so DVE never
                        # stalls waiting on this chunk's Pool fold1
                        pend2.append((mh, M, dst))
                        if len(pend2) > 1:
                            do_tail(*pend2.pop(0))

                    def do_tail(mh, M, dst):
                        m2 = m2p.tile([128, 272], BF16, tag="m2")
                        nc.vector.tensor_tensor(
                            out=ap(m2, 0, M, 0, [[17, 16], [1, 17]]),
                            in0=ap(mh, 0, M, 0, [[34, 16], [1, 17]]),
                            in1=ap(mh, 0, M, 17, [[34, 16], [1, 17]]),
                            op=AluOpType.add)
                        at = tp.tile([128, 16], F32, tag="at")
                        nc.vector.tensor_reduce(
                            at[:M, :], ap(m2, 0, M, 0, [[17, 16], [1, 17]]),
                            AX.X, AluOpType.add)
                        pend.append((at, M, dst))

                    def emit_transpose(at, M):
                        if state["ptt"] is None:
                            pttt = pttp.tile([128, 512], F32, tag="ptt")
                            state["ptt"] = pttt
                            state["off"] = 0
                        nc.tensor.transpose(
                            ap(state["ptt"], 0, 16, state["off"], [[1, M]]),
                            at[:M, :], ident[:M, :M].bitcast(F32))
                        state["off"] += M

                    def drain_pend(keep):
                        while len(pend) > keep:
                            at, M, dst = pend.pop(0)
                            emit_transpose(at, M)
                            if dst is not None:
                                nc.scalar.activation(
                                    dst,
                                    ap(state["ptt"], 0, 16, 0,
                                       [[1, state["off"]]]),
                                    AF.Copy)
                                state["ptt"] = None

                    tiles = alloc_head()
                    for _ in prologue_steps(0, tiles):
                        pass
                    for hi in range(4):
                        A0 = 16 * hi
                        t = tiles
                        if hi < 3:
                            tiles = alloc_head()
                            nxt = prologue_steps(hi + 1, tiles)
                        else:
                            nxt = pe_p_steps()
                        er, ertail = t["er"], t["ertail"]
                        for wb in range(32):
                            dst = (ap(A_sb, A0, 16, 2 * (wb - 3) * H,
                                      [[H, 8], [1, 64]])
                                   if wb % 4 == 3 else None)
                            do_chunk(t, wb,
                                     ap(t["ecs"], 0, 65, wb * 128, [[1, 128]]),
                                     ap(t["er2"], 0, 128, 2 * wb * 66,
                                        [[0, 16], [1, H]]),
                                     wb, 128, dst)
                            drain_pend(10)
                            next(nxt, None)
                        do_chunk(t, 32,
                                 ap(t["ecs"], 0, 65, 64 * 64, [[1, 64]]),
                                 ap(t["er2"], 0, 64, 64 * 66,
                                    [[0, 16], [1, H]]),
                                 32, 64, ap(A_sb, A0, 16, 64 * H, [[1, 64]]))
                        drain_pend(10)
                        next(nxt, None)
                        do_chunk(t, 33,
                                 ap(t["ectail"], 0, 65, 0, [[1, H]]),
                                 ap(ertail, 0, 65, 0, [[0, 16], [1, H]]),
                                 33, 65, ap(A_sb, A0, 16, 64, [[H, H]]))
                        drain_pend(10)
                        for _ in nxt:
                            pass
                    drain_pend(0)

            # ===== S8: dconv(A) + dconv(P) -> dc3 rows 0:32 =====
            with (
                tc.tile_pool(name="dcpool", bufs=1) as dcp,
                tc.tile_pool(name="ps8", bufs=2, space="PSUM") as ps8,
            ):
                dc3 = dcp.tile([96, IMGP * IMGP], BF16, tag="dc3")
                # zero borders: block b holds dcpad rows shifted by b, so
                # block0 rows {0,129}, block1 rows {128,129-ish}, block2
                # rows {127,128}; plus the 1-px column strips everywhere.
                nc.gpsimd.dma_start(out=ap(dc3, 0, 32, 0, [[1, IMGP]]),
                                    in_=zbf[:32, :IMGP])
                nc.sync.dma_start(
                    out=ap(dc3, 0, 32, 129 * IMGP, [[1, IMGP]]),
                    in_=zbf[:32, :IMGP])
                nc.sync.dma_start(
                    out=ap(dc3, 32, 32, 128 * IMGP, [[1, 2 * IMGP]]),
                    in_=zbf[:32, :2 * IMGP])
                nc.sync.dma_start(
                    out=ap(dc3, 64, 32, 127 * IMGP, [[1, 3 * IMGP]]),
                    in_=zbf[:32, :3 * IMGP])
                for blk in range(3):
                    nc.sync.dma_start(
                        out=ap(dc3, 32 * blk, 32, IMGP, [[IMGP, 128], [1, 1]]),
                        in_=zbf[:32, :128])
                    nc.sync.dma_start(
                        out=ap(dc3, 32 * blk, 32, IMGP + 129,
                               [[IMGP, 128], [1, 1]]),
                        in_=zbf[:32, :128])
                # a0-outer so dc rows complete in ascending order; the
                # row-shifted copies for the 96-deep final conv are issued
                # piecewise so S9 can pipeline behind S8.
                shift_done = 0

                def dc3_shift_upto(row):
                    nonlocal shift_done
                    lo = shift_done
                    if row <= lo:
                        return
                    nc.sync.dma_start(
                        out=dc3[32:64, lo * IMGP:row * IMGP],
                        in_=dc3[0:32, (lo + 1) * IMGP:(row + 1) * IMGP])
                    nc.gpsimd.dma_start(
                        out=dc3[64:96, lo * IMGP:row * IMGP],
                        in_=dc3[0:32, (lo + 2) * IMGP:(row + 2) * IMGP])
                    shift_done = row

                for a0 in range(0, 64, 8):
                    for pr in range(2):
                        for ps in range(2):
                            pt = ps8.tile([32, 512], F32, tag="dcps")
                            w0 = (pr * 2 + ps) * 32
                            nc.tensor.matmul(
                                pt[:, :], Wdc[:, w0:w0 + 32],
                                ap(A_sb, 0, 128, ps * H + pr + a0,
                                   [[1, 8], [H, 64]]),
                                start=True, stop=True)
                            dst = ap(dc3, 0, 32,
                                     (2 * a0 + pr + 1) * IMGP + ps + 1,
                                     [[2 * IMGP, 8], [2, 64]])
                            if (pr * 2 + ps) % 2 == 0:
                                nc.scalar.activation(dst, pt[:, :],
                                                     AF.Identity,
                                                     bias=bdc[:, :])
                            else:
                                nc.vector.scalar_tensor_tensor(
                                    out=dst, in0=pt[:, :], scalar=1.0,
                                    in1=ap(bdc, 0, 32, 0, [[0, 8], [0, 64]]),
                                    op0=AluOpType.mult, op1=AluOpType.add)
                    if a0 in (24, 40, 56):
                        # rows complete up to 2*a0+16 after this block
                        dc3_shift_upto(2 * a0 + 14)
                nc.sync.dma_start(
                    out=dc3[32:64, shift_done * IMGP:IMGP * IMGP - IMGP],
                    in_=dc3[0:32, (shift_done + 1) * IMGP:])
                nc.gpsimd.dma_start(
                    out=dc3[64:96, shift_done * IMGP:IMGP * IMGP - 2 * IMGP],
                    in_=dc3[0:32, (shift_done + 2) * IMGP:IMGP * IMGP])

                # ===== S9: final conv partial, 96-deep =====
                with (
                    tc.tile_pool(name="opool", bufs=2) as op_,
                    tc.tile_pool(name="ps9", bufs=2, space="PSUM") as ps9,
                ):
                    ost = None
                    for r0 in range(0, IMG, 4):
                        pt = ps9.tile([64, 512], F32, tag="o")
                        for kx in range(3):
                            rhs = ap(dc3, 0, 96, r0 * IMGP + kx,
                                     [[IMGP, 4], [1, IMG]])
                            nc.tensor.matmul(pt[:, :],
                                             Wout3[:, kx * 64:kx * 64 + 64],
                                             rhs, start=(kx == 0),
                                             stop=(kx == 2))
                        if r0 % 16 == 0:
                            ost = op_.tile([64, 2048], BF16, tag="ost")
                        sl = (r0 % 16) // 4 * 512
                        if (r0 // 4) % 2 == 0:
                            nc.scalar.activation(ost[:, sl:sl + 512], pt[:, :],
                                                 AF.Identity, bias=bfin[:, :])
                        else:
                            nc.vector.scalar_tensor_tensor(
                                out=ost[:, sl:sl + 512], in0=pt[:, :],
                                scalar=1.0,
                                in1=ap(bfin, 0, 64, 0, [[0, 512]]),
                                op0=AluOpType.mult, op1=AluOpType.add)
                        if r0 % 16 == 12:
                            oeng = nc.sync if r0 % 32 == 12 else nc.gpsimd
                            oeng.dma_start(out=out_d[:, r0 - 12:r0 + 4, :],
                                           in_=ost[:, :])

    nc.compile()
    return nc


_NC_CACHE = None


def kernel(**inputs):
    global _NC_CACHE
    if _NC_CACHE is None:
        _NC_CACHE = build_nc()
    nc = _NC_CACHE
    in_maps = [prep_core_inputs(inputs, c // 2, c % 2) for c in range(8)]
    res = run_bass_kernel_spmd(nc, in_maps, list(range(8)))
    out = np.zeros((4, 64, IMG, IMG), np.float32)
    for b in range(4):
        out[b] = (res.results[2 * b]["out"].astype(np.float32) +
                  res.results[2 * b + 1]["out"].astype(np.float32))
    return out

